# revision 6
# baseline (speedup 1.0000x reference)
"""Distributed GQA attention prefill for TRN2 (8 NeuronCores).

Problem: T=2048, D=4096, N=32 query heads, K=8 kv heads, H=128.
    q = x @ w_q; k = x @ w_k; v = x @ w_v   (fused in the reference)
    rope(q), rope(k); causal GQA attention; out = o @ w_o

Sharding (tensor-parallel over heads): core c owns query heads
4c..4c+3 and kv head c (GQA groups align). w_q/w_o sharded on N,
w_k/w_v on K, x replicated. Each core computes its partial o_proj
output [T, D]; a chunked bf16 ReduceScatter sums partials; the host
concatenates the per-core row shards.

Single software-pipelined phase: the T=2048 sequence is processed in
4 panels of 512. Block b emits QKV+RoPE for panel b interleaved (in
PE program order) with attention+o_proj for panel b-1, so the PE
stream stays dense. All 80 128x128 transposes (PV outputs + V) go
through the DMA XBAR (sync queue, SBUF->SBUF) instead of the PE,
saving ~20us of PE time and 3 PSUM banks (scores get 3 rotating
banks, which un-gates the exp-paced score matmul stream).

Startup: weights/x are split into 256-512KB pieces across the 3 DMA
queues and QKV matmul (output, x-quarter) pairs are emitted in
arrival order, so the PE starts ~11us after kernel entry and never
waits on a 1MB+ transfer.

ReduceScatter: chunks are sets of 128-row t-blocks (rows are placed
into the chunk tile by completion order, not global order), sized so
the serial CC chain never backs up: 2-t-block chunks triggered at the
2nd and 4th oproj of each panel. Panel 2's last two oproj units
(tb11, tb8) are deferred into block 4 and interleaved at matmul
granularity with panel 3's exp-gated score stream (ScalarE is the
bottleneck there: 64 exps ~41us); the final chunks are 1 t-block
(tb15, tb12) so the post-PE tail is ~2 small RS ops.

Device context (measured): GPIO/SW throttle pins the PE at 13/16 =
1.95GHz for whole runs; HAM re-throttles to 1.2GHz after any >3.4us
PE gap. LDWEIGHTS is fully hidden behind matmul streaming when warm
(measured 262ns spacing for 512-wide bf16 MMs, 68ns for 129-wide),
so many small matmuls are fine. Per-queue DMA bandwidth ~119GB/s.
Uncontended CC costs: RS ~11.6us + 4.5us/MB, tiny AllReduce ~9us.
Shared-HBM dram tensors are only shared within core pairs {2c,2c+1}
(one chip), so a full manual 8-way reduction is not possible.
"""

import numpy as np
import ml_dtypes

T, D, NH, KH, H = 2048, 4096, 32, 8, 128
THETA = 10000.0
G = NH // KH          # 4 query heads per core
N_CORES = 8
TP_SIZE = 512         # t-panel
NTP = T // TP_SIZE    # 4 panels
NTB = T // 128        # 16 t/s blocks
NDB = D // 128        # 32 d blocks
NQ = 4                # x quarters per panel (8 d-blocks each)
QDB = NDB // NQ
SCALE = 1.0 / float(np.sqrt(H))
VEXT_STRIDE = 160     # v_ext row stride (129 used; 320B so XBAR
                      # transpose dsts stay 64B-aligned)
JSEQ = [1, 2, 3, 0]   # t-block order within a panel

# ReduceScatter chunks: lists of t-blocks, in completion order given
# JSEQ. Chunk rows are packed in list order (idx*128), not global row
# order; assemble_output unpacks. Panel p completes tb 4p+1, 4p+2,
# 4p+3, 4p+0; panel 2's last two (tb11, tb8) are deferred to block 4.
RS_CHUNKS_TB = [[1, 2], [3, 0], [5, 6], [7, 4], [9, 10], [11, 8],
                [13, 14], [15], [12]]
TB2CHUNK = {tb: (ch, idx) for ch, tbs in enumerate(RS_CHUNKS_TB)
            for idx, tb in enumerate(tbs)}
CHUNK_ROWS = [128 * len(tbs) for tbs in RS_CHUNKS_TB]
CHUNK_OUT_OFF = np.concatenate(
    [[0], np.cumsum([n // N_CORES for n in CHUNK_ROWS])]).tolist()

_NC_CACHE = {}


def _enable_ldw_opt():
    """No-op kept for test.py compatibility (LDWEIGHTS is hidden by
    the PE's 64-deep reorder window when warm; no opt needed)."""
    return


def _build_nc():
    import concourse.mybir as mybir
    import concourse.tile as tile
    from concourse import bacc

    BF16 = mybir.dt.bfloat16
    F32 = mybir.dt.float32
    EXP = mybir.ActivationFunctionType.Exp

    nc = bacc.Bacc("TRN2", target_bir_lowering=False, debug=False,
                   num_devices=N_CORES)

    xt_ext = nc.dram_tensor("xt", [128, NTP, NDB, TP_SIZE], BF16,
                            kind="ExternalInput")
    wq_ext = nc.dram_tensor("wq", [128, G, NDB, H], BF16,
                            kind="ExternalInput")
    wk_ext = nc.dram_tensor("wk", [128, NDB, H], BF16, kind="ExternalInput")
    wv_ext = nc.dram_tensor("wv", [128, NDB, H], BF16, kind="ExternalInput")
    wo_ext = nc.dram_tensor("wo", [128, G, D], BF16, kind="ExternalInput")
    cos_ext = nc.dram_tensor("cos_t", [H, T], BF16, kind="ExternalInput")
    sin_ext = nc.dram_tensor("sin_t", [H, T], BF16, kind="ExternalInput")
    mask_ext = nc.dram_tensor("maskp", [128, 128], BF16, kind="ExternalInput")
    out_ext = nc.dram_tensor("out", [T // N_CORES, D], BF16,
                             kind="ExternalOutput")

    with tile.TileContext(nc) as tc:
        with (
            tc.tile_pool(name="consts", bufs=1) as consts,
            tc.tile_pool(name="persist", bufs=1) as persist,
            tc.tile_pool(name="xqp", bufs=8) as xqp,
            tc.tile_pool(name="qtp", bufs=2) as qtp,
            tc.tile_pool(name="csp", bufs=2) as csp,
            tc.tile_pool(name="ptp", bufs=30) as ptp,
            tc.tile_pool(name="ropep", bufs=1) as ropep,
            tc.tile_pool(name="scp", bufs=4) as scp,
            tc.tile_pool(name="osbp", bufs=2) as osbp,
            tc.tile_pool(name="qkvps", bufs=1, space="PSUM") as qkv_ps,
            tc.tile_pool(name="sps", bufs=3, space="PSUM") as sps,
            tc.tile_pool(name="smallps", bufs=2, space="PSUM") as smallps,
            tc.tile_pool(name="oprojps", bufs=2, space="PSUM") as oprojps,
            tc.tile_pool(name="dram", bufs=1, space="DRAM") as dram,
        ):
            wq_sb = consts.tile([128, G, NDB, H], BF16)
            wk_sb = consts.tile([128, NDB, H], BF16)
            wv_sb = consts.tile([128, NDB, H], BF16)
            wo_sb = consts.tile([128, G, D], BF16)
            mask_sb = consts.tile([128, 128], BF16)

            kT = persist.tile([128, T], BF16)
            v_ext = persist.tile([128, NTB, VEXT_STRIDE], BF16)

            rs_in = [dram.tile([n, D], BF16, tag=f"rsw{ch}", name=f"rsw{ch}")
                     for ch, n in enumerate(CHUNK_ROWS)]
            rs_out = [dram.tile([n // N_CORES, D], BF16, tag=f"rso{ch}",
                                name=f"rso{ch}")
                      for ch, n in enumerate(CHUNK_ROWS)]

            nc.vector.memset(v_ext[:, :, 128:129], 1.0)

            # mutable emission state
            state = {
                "xq": {},      # (panel, quarter) -> sbuf tile
                "cs": {},      # panel -> (cos, sin) sbuf tiles
                "qT": {},      # (panel, g) -> roped qT tile [128, 512]
                "pts": {},     # (g, sb) -> (tile, col0) P^T tiles of cur panel
                "oT": {},      # g -> (tile, panel) of cur att panel
                "rs_done": {},
            }

            def fetch_x(p, queues=None):
                qs = queues or [nc.sync] * NQ
                for q in range(NQ):
                    xq = xqp.tile([128, QDB, TP_SIZE], BF16, tag="xq",
                                  name=f"xq{p}_{q}")
                    qs[q].dma_start(
                        out=xq[:],
                        in_=xt_ext[:, p, q * QDB:(q + 1) * QDB, :])
                    state["xq"][(p, q)] = xq

            def fetch_cs(p):
                tsl = slice(p * TP_SIZE, (p + 1) * TP_SIZE)
                cos_sb = csp.tile([H, TP_SIZE], BF16, tag="cos",
                                  name=f"cos{p}")
                sin_sb = csp.tile([H, TP_SIZE], BF16, tag="sin",
                                  name=f"sin{p}")
                nc.gpsimd.dma_start(out=cos_sb[:], in_=cos_ext[:, tsl])
                nc.gpsimd.dma_start(out=sin_sb[:], in_=sin_ext[:, tsl])
                state["cs"][p] = (cos_sb, sin_sb)

            def rope(p, raw, dst):
                """dst = raw*cos + halfswap(raw)*sin for panel p [128,512]."""
                cos_sb, sin_sb = state["cs"][p]
                sw = ropep.tile([128, TP_SIZE], BF16, tag="ropesw",
                                name=f"sw{p}")
                t1 = ropep.tile([128, TP_SIZE], BF16, tag="ropet1",
                                name=f"t1{p}")
                nc.scalar.dma_start(out=sw[0:64, :], in_=raw[64:128, :])
                nc.scalar.dma_start(out=sw[64:128, :], in_=raw[0:64, :])
                nc.vector.tensor_tensor(out=t1[:], in0=raw[:], in1=cos_sb[:],
                                        op=mybir.AluOpType.mult)
                nc.vector.tensor_tensor(out=sw[:], in0=sw[:], in1=sin_sb[:],
                                        op=mybir.AluOpType.mult)
                nc.vector.tensor_tensor(out=dst[:], in0=t1[:], in1=sw[:],
                                        op=mybir.AluOpType.add)

            def qkv_unit(p, which):
                """One QKV output for panel p: 'k' | 'v' | 0..G-1."""
                ps = qkv_ps.tile([128, TP_SIZE], F32, tag="qkv",
                                 name=f"qkv{p}_{which}")
                if which == "k":
                    w = wk_sb
                elif which == "v":
                    w = wv_sb
                else:
                    w = wq_sb[:, which]
                for db in range(NDB):
                    xq = state["xq"][(p, db // QDB)]
                    nc.tensor.matmul(
                        ps[:], w[:, db, :], xq[:, db % QDB, :],
                        start=(db == 0), stop=(db == NDB - 1))
                tsl = slice(p * TP_SIZE, (p + 1) * TP_SIZE)
                if which == "k":
                    nc.scalar.copy(kT[:, tsl], ps[:])
                    rope(p, kT[:, tsl], kT[:, tsl])
                elif which == "v":
                    vraw = scp.tile([128, TP_SIZE], BF16, tag="vraw",
                                    bufs=1, name=f"vraw{p}")
                    nc.scalar.copy(vraw[:], ps[:])
                    for j in range(4):
                        sb = 4 * p + j
                        nc.sync.dma_start(
                            out=v_ext[:, sb, 0:128],
                            in_=vraw[:, j * 128:(j + 1) * 128],
                            transpose=True)
                else:
                    g = which
                    qt = qtp.tile([128, TP_SIZE], BF16, tag=f"qT{g}",
                                  name=f"qT{p}_{g}")
                    nc.vector.tensor_copy(qt[:], ps[:])
                    rope(p, qt[:], qt[:])
                    state["qT"][(p, g)] = qt

            def scores_mm(p, g, sb):
                """One score block matmul + exp + mask for (p, g, sb)."""
                qt = state["qT"][(p, g)]
                jj = sb - 4 * p
                c0 = max(jj, 0) * 128
                w = TP_SIZE - c0
                ps_s = sps.tile([128, TP_SIZE], F32, tag="s",
                                name=f"s{p}_{g}_{sb}")
                nc.tensor.matmul(
                    ps_s[:, 0:w], kT[:, sb * 128:(sb + 1) * 128],
                    qt[:, c0:TP_SIZE], start=True, stop=True)
                pt = ptp.tile([128, TP_SIZE], BF16, tag="pt",
                              name=f"pt{p}_{g}_{sb}")
                nc.scalar.activation(pt[:, 0:w], ps_s[:, 0:w], EXP,
                                     scale=SCALE)
                if jj >= 0:
                    nc.vector.tensor_tensor(
                        out=pt[:, 0:128], in0=pt[:, 0:128], in1=mask_sb[:],
                        op=mybir.AluOpType.mult)
                state["pts"][(p, g, sb)] = (pt, c0)

            def scores_unit(p, g):
                for sb in range(4 * p + 4):
                    scores_mm(p, g, sb)

            def pv_mm(p, g, j):
                """PV matmuls + DVE normalize for t-block j of head g."""
                if g not in state["oT"] or state["oT"][g][1] != p:
                    oT = scp.tile([128, TP_SIZE], BF16, tag=f"oT{g}",
                                  bufs=1, name=f"oT{p}_{g}")
                    state["oT"][g] = (oT, p)
                tb = 4 * p + j
                ps_pv = smallps.tile([128, 132], F32, tag="sm",
                                     name=f"pv{p}_{g}_{j}")
                for sb in range(tb + 1):
                    pt, c0 = state["pts"][(p, g, sb)]
                    lo = j * 128 - c0
                    nc.tensor.matmul(
                        ps_pv[:, 0:129], pt[:, lo:lo + 128],
                        v_ext[:, sb, 0:129],
                        start=(sb == 0), stop=(sb == tb),
                        skip_group_check=True)
                rc = scp.tile([128, 1], F32, tag="rc", bufs=4,
                              name=f"rc{p}_{g}_{j}")
                nc.vector.reciprocal(rc[:], ps_pv[:, 128:129])
                ob = scp.tile([128, 128], BF16, tag="ob",
                              bufs=4, name=f"ob{p}_{g}_{j}")
                nc.vector.tensor_scalar_mul(ob[:], ps_pv[:, 0:128], rc[:])
                # normalized block -> oT via DMA XBAR transpose (sync q)
                oT = state["oT"][g][0]
                nc.sync.dma_start(out=oT[:, j * 128:(j + 1) * 128],
                                  in_=ob[:], transpose=True)

            def pv_unit(p, g, js=JSEQ):
                for j in js:
                    pv_mm(p, g, j)

            def oproj_dq(p, j, dq, force_dve=False):
                """One quarter of o_proj for t-block j of panel p."""
                tb = 4 * p + j
                ch, idx = TB2CHUNK[tb]
                row = idx * 128
                last = p == NTP - 1
                osb = osbp.tile([128, D // 4], BF16, tag="osb",
                                name=f"osb{tb}_{dq}")
                for dp in range(2):
                    od = oprojps.tile([128, 512], F32, tag="od",
                                      name=f"od{tb}_{dq}_{dp}")
                    dc = dq * 2 + dp
                    for g in range(G):
                        nc.tensor.matmul(
                            od[:],
                            state["oT"][g][0][:, j * 128:(j + 1) * 128],
                            wo_sb[:, g, dc * 512:(dc + 1) * 512],
                            start=(g == 0), stop=(g == G - 1),
                            skip_group_check=True)
                    eng = 1 if (last or force_dve) else dq % 2
                    if eng == 0:
                        nc.scalar.copy(
                            osb[:, dp * 512:(dp + 1) * 512], od[:])
                    else:
                        nc.vector.tensor_copy(
                            osb[:, dp * 512:(dp + 1) * 512], od[:])
                q = nc.sync if (last or force_dve) else nc.scalar
                q.dma_start(
                    out=rs_in[ch][row:row + 128,
                                  dq * 1024:(dq + 1) * 1024],
                    in_=osb[:])

            def rs_maybe_trigger(tb):
                ch, _ = TB2CHUNK[tb]
                state["rs_done"].setdefault(ch, 0)
                state["rs_done"][ch] += 1
                if state["rs_done"][ch] == len(RS_CHUNKS_TB[ch]):
                    nc.gpsimd.collective_compute(
                        "ReduceScatter",
                        mybir.AluOpType.add,
                        replica_groups=[list(range(N_CORES))],
                        ins=[rs_in[ch].opt()],
                        outs=[rs_out[ch].opt()],
                    )
                    o0, o1 = CHUNK_OUT_OFF[ch], CHUNK_OUT_OFF[ch + 1]
                    nc.gpsimd.dma_start(
                        out=out_ext[o0:o1, :], in_=rs_out[ch][:])

            def oproj_unit(p, j):
                for dq in range(4):
                    oproj_dq(p, j, dq)
                rs_maybe_trigger(4 * p + j)

            # ---- block 0: QKV panel 0, DMA-arrival-ordered ----
            # Startup pieces are 256-512KB so the PE starts ~11us in.
            # Queue sequences (512KB ~ 4.3us at ~119GB/s):
            #  sync:   xq0[0]a, xq0[0]b, xq0[1], xq0[2], xq0[3],
            #          wq2_a, wq2_b, xp1[0], xp1[1]
            #  scalar: wk[0:8], wk[8:16], wk[16:32], wq0_a, wq0_b,
            #          wq3_a, wq3_b, xp1[2], xp1[3]
            #  gpsimd: mask, wv_a, wv_b, cs0, wq1_a, wq1_b, wo, cs1
            xq0 = {}
            for q in range(NQ):
                xq0[q] = xqp.tile([128, QDB, TP_SIZE], BF16, tag="xq",
                                  name=f"xq0_{q}")
                state["xq"][(0, q)] = xq0[q]
            nc.gpsimd.dma_start(out=mask_sb[:], in_=mask_ext[:])
            nc.sync.dma_start(out=xq0[0][:, 0:4, :], in_=xt_ext[:, 0, 0:4, :])
            nc.scalar.dma_start(out=wk_sb[:, 0:8], in_=wk_ext[:, 0:8])
            nc.gpsimd.dma_start(out=wv_sb[:, 0:16], in_=wv_ext[:, 0:16])
            nc.sync.dma_start(out=xq0[0][:, 4:QDB, :],
                              in_=xt_ext[:, 0, 4:QDB, :])
            nc.scalar.dma_start(out=wk_sb[:, 8:16], in_=wk_ext[:, 8:16])
            nc.gpsimd.dma_start(out=wv_sb[:, 16:32], in_=wv_ext[:, 16:32])
            nc.sync.dma_start(out=xq0[1][:], in_=xt_ext[:, 0, QDB:2 * QDB, :])
            nc.scalar.dma_start(out=wk_sb[:, 16:32], in_=wk_ext[:, 16:32])
            fetch_cs(0)
            nc.sync.dma_start(out=xq0[2][:],
                              in_=xt_ext[:, 0, 2 * QDB:3 * QDB, :])
            nc.scalar.dma_start(out=wq_sb[:, 0, 0:16], in_=wq_ext[:, 0, 0:16])
            nc.gpsimd.dma_start(out=wq_sb[:, 1, 0:16], in_=wq_ext[:, 1, 0:16])
            nc.sync.dma_start(out=xq0[3][:],
                              in_=xt_ext[:, 0, 3 * QDB:4 * QDB, :])
            nc.scalar.dma_start(out=wq_sb[:, 0, 16:32],
                                in_=wq_ext[:, 0, 16:32])
            nc.gpsimd.dma_start(out=wq_sb[:, 1, 16:32],
                                in_=wq_ext[:, 1, 16:32])
            nc.sync.dma_start(out=wq_sb[:, 2, 0:16], in_=wq_ext[:, 2, 0:16])
            nc.scalar.dma_start(out=wq_sb[:, 3, 0:16], in_=wq_ext[:, 3, 0:16])
            nc.sync.dma_start(out=wq_sb[:, 2, 16:32], in_=wq_ext[:, 2, 16:32])
            nc.scalar.dma_start(out=wq_sb[:, 3, 16:32],
                                in_=wq_ext[:, 3, 16:32])
            nc.gpsimd.dma_start(out=wo_sb[:], in_=wo_ext[:])
            # x panel 1 + cos/sin panel 1 prefetch
            fetch_x(1, queues=[nc.sync, nc.sync, nc.scalar, nc.scalar])
            fetch_cs(1)

            outs0 = ["k", "v", 0, 1, 2, 3]
            pools0 = [sps, sps, qkv_ps, sps, oprojps, oprojps]
            tags0 = ["s", "s", "qkv", "s", "od", "od"]
            ps0 = {}
            for o, pool, tg in zip(outs0, pools0, tags0):
                ps0[o] = pool.tile([128, TP_SIZE], F32, tag=tg,
                                   name=f"qkv0_{o}")
            # (output, quarter) pairs in DMA arrival order; "0a"/"0b"
            # are the two halves of quarter 0 (first k matmuls ~11us).
            PAIR_ORDER = [("k", "0a"), ("k", "0b"), ("v", 0), ("k", 1),
                          ("v", 1), (0, 0), ("k", 2), ("v", 2),
                          (1, 0), (1, 1), ("k", 3), ("v", 3),
                          (0, 1), (0, 2), (0, 3), (1, 2), (1, 3),
                          (2, 0), (2, 1), (3, 0), (3, 1),
                          (2, 2), (2, 3), (3, 2), (3, 3)]
            done = {o: 0 for o in outs0}
            for o, q in PAIR_ORDER:
                if q == "0a":
                    dbs = range(0, 4)
                elif q == "0b":
                    dbs = range(4, 8)
                else:
                    dbs = range(q * QDB, (q + 1) * QDB)
                w = (wk_sb if o == "k" else
                     wv_sb if o == "v" else wq_sb[:, o])
                for db in dbs:
                    nc.tensor.matmul(
                        ps0[o][:], w[:, db, :],
                        state["xq"][(0, db // QDB)][:, db % QDB, :],
                        start=(done[o] == 0),
                        stop=(done[o] == NDB - 1),
                        skip_group_check=True)
                    done[o] += 1
            # copy-outs, ropes, v transpose for panel 0
            nc.scalar.copy(kT[:, 0:TP_SIZE], ps0["k"][:])
            rope(0, kT[:, 0:TP_SIZE], kT[:, 0:TP_SIZE])
            vraw = scp.tile([128, TP_SIZE], BF16, tag="vraw", bufs=1,
                            name="vraw0")
            nc.scalar.copy(vraw[:], ps0["v"][:])
            for g in range(G):
                qt = qtp.tile([128, TP_SIZE], BF16, tag=f"qT{g}",
                              name=f"qT0_{g}")
                nc.scalar.copy(qt[:], ps0[g][:])
                rope(0, qt[:], qt[:])
                state["qT"][(0, g)] = qt
            for j in range(4):
                nc.sync.dma_start(out=v_ext[:, j, 0:128],
                                  in_=vraw[:, j * 128:(j + 1) * 128],
                                  transpose=True)

            # ---- blocks 1..3: att(b-1) + oproj interleaved with QKV(b) --
            for b in range(1, NTP):
                p = b - 1
                qkv = [lambda w=w, b=b: qkv_unit(b, w)
                       for w in ["k", "v", 0, 1, 2, 3]]
                att = [lambda p=p: scores_unit(p, 0)]
                for g in range(G - 1):
                    def pv_sc(g=g, p=p):
                        pv_unit(p, g)
                        scores_unit(p, g + 1)
                    att.append(pv_sc)
                # g3 + oproj tail; for p==2 defer tb11 (j=3) and tb8
                # (j=0) oproj into block 4.
                oj = JSEQ if p < 2 else JSEQ[:2]

                def tail_u(i, p=p, oj=oj):
                    pv_mm(p, G - 1, JSEQ[i])
                    if i >= 1 and i - 1 < len(oj):
                        oproj_unit(p, oj[i - 1])
                att.append(lambda: tail_u(0))
                att.append(lambda: tail_u(1))
                att.append(lambda: tail_u(2))
                att.append(lambda: tail_u(3))
                if len(oj) == 4:
                    att.append(lambda p=p: oproj_unit(p, JSEQ[3]))
                n_u = max(len(att), len(qkv))
                for i in range(n_u):
                    if i < len(att):
                        att[i]()
                    if i < len(qkv):
                        qkv[i]()
                    if i == 0 and b + 1 < NTP:
                        fetch_cs(b + 1)
                    if i == 2 and b + 1 < NTP:
                        fetch_x(b + 1)

            # ---- block 4: att(panel 3) + deferred oproj(p2) ----
            # ScalarE is the bottleneck in the score phase (64 exps
            # ~41us vs 17us of score MMs), so filler PE work (deferred
            # p2 oproj quarters, then pv units) is interleaved into
            # the exp-gated score stream at ~per-3-MMs granularity.
            p = 3
            fillers = []
            for j in (JSEQ[2], JSEQ[3]):   # tb11 then tb8
                for dq in range(4):
                    fillers.append(
                        lambda j=j, dq=dq: oproj_dq(2, j, dq,
                                                    force_dve=True))
                fillers.append(lambda j=j: rs_maybe_trigger(8 + j))
            fill_i = 0
            mm_cnt = 0
            for g in range(G):
                for sb in range(4 * p + 4):
                    scores_mm(p, g, sb)
                    mm_cnt += 1
                    if mm_cnt % 3 == 0 and fill_i < len(fillers):
                        fillers[fill_i]()
                        fill_i += 1
                if g >= 1:
                    for j in JSEQ:
                        fillers.append(lambda g=g, j=j: pv_mm(p, g - 1, j))
            while fill_i < len(fillers):
                fillers[fill_i]()
                fill_i += 1
            # tail: pv(g3, j) then oproj(j) staggered by one
            pv_mm(p, G - 1, JSEQ[0])
            pv_mm(p, G - 1, JSEQ[1])
            oproj_unit(p, JSEQ[0])
            pv_mm(p, G - 1, JSEQ[2])
            oproj_unit(p, JSEQ[1])
            pv_mm(p, G - 1, JSEQ[3])
            oproj_unit(p, JSEQ[2])
            oproj_unit(p, JSEQ[3])

    nc.compile()
    return nc


def get_nc():
    if "nc" not in _NC_CACHE:
        _NC_CACHE["nc"] = _build_nc()
    return _NC_CACHE["nc"]


def make_in_maps(x, positions, w_q, w_k, w_v, w_o):
    """Host-side sharding + RoPE table / mask precompute."""
    x = np.ascontiguousarray(np.asarray(x, np.float32))
    positions = np.asarray(positions)

    half = H // 2
    inv_freq = 1.0 / (THETA ** (np.arange(half, dtype=np.float32) / half))
    ang = positions.astype(np.float32)[:, None] * inv_freq[None, :]  # [T, 64]
    cos = np.cos(ang)   # [T, 64]
    sin = np.sin(ang)
    cos_t = np.empty((H, T), np.float32)
    sin_t = np.empty((H, T), np.float32)
    cos_t[0:half] = cos.T
    cos_t[half:] = cos.T
    sin_t[0:half] = -sin.T
    sin_t[half:] = sin.T
    cos_t = cos_t.astype(ml_dtypes.bfloat16)
    sin_t = sin_t.astype(ml_dtypes.bfloat16)

    # mask[s, t] = 1 if s <= t (lower-left of P^T allowed region)
    idx = np.arange(128)
    maskp = (idx[:, None] <= idx[None, :]).astype(ml_dtypes.bfloat16)

    xt = x.astype(ml_dtypes.bfloat16).T  # [D, T]
    xt4 = np.ascontiguousarray(
        xt.reshape(NDB, 128, NTP, TP_SIZE).transpose(1, 2, 0, 3))
    w_q = np.asarray(w_q, np.float32).reshape(D, NH, H).astype(
        ml_dtypes.bfloat16)
    w_k = np.asarray(w_k, np.float32).reshape(D, KH, H).astype(
        ml_dtypes.bfloat16)
    w_v = np.asarray(w_v, np.float32).reshape(D, KH, H).astype(
        ml_dtypes.bfloat16)
    w_o = np.asarray(w_o, np.float32).reshape(NH, H, D).astype(
        ml_dtypes.bfloat16)

    def blk(w):
        """[D, n] -> [128, NDB, n] with row d = a*128 + p."""
        return np.ascontiguousarray(
            w.reshape(NDB, 128, -1).transpose(1, 0, 2))

    in_maps = []
    for c in range(N_CORES):
        # wq g-major: [128, G, NDB, H] so each head's chunk is contiguous
        wq_c = w_q[:, G * c:G * (c + 1), :]            # [D, G, H]
        wq_blk = np.ascontiguousarray(
            wq_c.reshape(NDB, 128, G, H).transpose(1, 2, 0, 3))
        in_maps.append({
            "xt": xt4,
            "wq": wq_blk,
            "wk": blk(w_k[:, c, :]),
            "wv": blk(w_v[:, c, :]),
            "wo": np.ascontiguousarray(
                w_o[G * c:G * (c + 1)].reshape(G, 128, D)
                .transpose(1, 0, 2)),
            "cos_t": cos_t,
            "sin_t": sin_t,
            "maskp": maskp,
        })
    return in_maps


def assemble_output(results):
    """results: list of 8 per-core dicts with 'out' [T//8, D] bf16.

    Chunk ch rows are packed t-block-list-major; the RS gave core c
    chunk-tile rows [c*k, (c+1)*k) where k = chunk_rows/8.
    """
    out = np.empty((T, D), np.float32)
    for c in range(N_CORES):
        o = np.asarray(results[c]["out"], np.float32)
        for ch, tbs in enumerate(RS_CHUNKS_TB):
            k = CHUNK_ROWS[ch] // N_CORES
            piece = o[CHUNK_OUT_OFF[ch]:CHUNK_OUT_OFF[ch + 1]]
            for r in range(k):
                cr = c * k + r
                tb = tbs[cr // 128]
                out[tb * 128 + (cr % 128)] = piece[r]
    return out


def kernel(x, positions, w_q, w_k, w_v, w_o):
    from concourse.bass_utils import run_bass_kernel_spmd

    _enable_ldw_opt()
    nc = get_nc()
    in_maps = make_in_maps(x, positions, w_q, w_k, w_v, w_o)
    res = run_bass_kernel_spmd(nc, in_maps, core_ids=list(range(N_CORES)))
    return assemble_output(res.results)


# revision 8
# speedup vs baseline: 1.1724x; 1.1724x over previous
"""Distributed GQA attention prefill for TRN2 (8 NeuronCores).

Problem: T=2048, D=4096, N=32 query heads, K=8 kv heads, H=128.
    q = x @ w_q; k = x @ w_k; v = x @ w_v   (fused in the reference)
    rope(q), rope(k); causal GQA attention; out = o @ w_o

Sharding (tensor-parallel over heads): core c owns query heads
4c..4c+3 and kv head c (GQA groups align). w_q/w_o sharded on N,
w_k/w_v on K, x replicated. Each core computes its partial o_proj
output [T, D]; a chunked bf16 ReduceScatter sums partials; the host
concatenates the per-core row shards.

Single software-pipelined phase: the T=2048 sequence is processed in
4 panels of 512. Block b emits QKV+RoPE for panel b interleaved (in
PE program order) with attention+o_proj for panel b-1, so the PE
stream stays dense. All 80 128x128 transposes (PV outputs + V) go
through the DMA XBAR (sync queue, SBUF->SBUF) instead of the PE,
saving ~20us of PE time and 3 PSUM banks (scores get 3 rotating
banks, which un-gates the exp-paced score matmul stream).

Startup: weights/x are split into 256-512KB pieces across the 3 DMA
queues and QKV matmul (output, x-quarter) pairs are emitted in
arrival order, so the PE starts ~11us after kernel entry and never
waits on a 1MB+ transfer.

ReduceScatter: chunks are sets of 128-row t-blocks (rows are placed
into the chunk tile by completion order, not global order), sized so
the serial CC chain never backs up: 2-t-block chunks triggered at the
2nd and 4th oproj of each panel. Panel 2's last two oproj units
(tb11, tb8) are deferred into block 4 and interleaved at matmul
granularity with panel 3's exp-gated score stream (ScalarE is the
bottleneck there: 64 exps ~41us); the final chunks are 1 t-block
(tb15, tb12) so the post-PE tail is ~2 small RS ops.

Device context (measured): GPIO/SW throttle pins the PE at 13/16 =
1.95GHz for whole runs; HAM re-throttles to 1.2GHz after any >3.4us
PE gap. LDWEIGHTS is fully hidden behind matmul streaming when warm
(measured 262ns spacing for 512-wide bf16 MMs, 68ns for 129-wide),
so many small matmuls are fine. Per-queue DMA bandwidth ~119GB/s.
Uncontended CC costs: RS ~11.6us + 4.5us/MB, tiny AllReduce ~9us.
Shared-HBM dram tensors are only shared within core pairs {2c,2c+1}
(one chip), so a full manual 8-way reduction is not possible.
"""

import numpy as np
import ml_dtypes

T, D, NH, KH, H = 2048, 4096, 32, 8, 128
THETA = 10000.0
G = NH // KH          # 4 query heads per core
N_CORES = 8
TP_SIZE = 512         # t-panel
NTP = T // TP_SIZE    # 4 panels
NTB = T // 128        # 16 t/s blocks
NDB = D // 128        # 32 d blocks
NQ = 4                # x quarters per panel (8 d-blocks each)
QDB = NDB // NQ
SCALE = 1.0 / float(np.sqrt(H))
VEXT_STRIDE = 160     # v_ext row stride (129 used; 320B so XBAR
                      # transpose dsts stay 64B-aligned)
JSEQ = [1, 2, 3, 0]   # t-block order within a panel

# ReduceScatter chunks: lists of t-blocks, in completion order given
# JSEQ. Chunk rows are packed in list order (idx*128), not global row
# order; assemble_output unpacks. Panel p completes tb 4p+1, 4p+2,
# 4p+3, 4p+0; panel 2's last two (tb11, tb8) are deferred to block 4.
RS_CHUNKS_TB = [[1, 2], [3, 0], [5, 6], [7, 4], [9, 10], [11, 8],
                [13, 14], [15], [12]]
TB2CHUNK = {tb: (ch, idx) for ch, tbs in enumerate(RS_CHUNKS_TB)
            for idx, tb in enumerate(tbs)}
CHUNK_ROWS = [128 * len(tbs) for tbs in RS_CHUNKS_TB]
CHUNK_OUT_OFF = np.concatenate(
    [[0], np.cumsum([n // N_CORES for n in CHUNK_ROWS])]).tolist()

_NC_CACHE = {}


def _enable_ldw_opt():
    """No-op kept for test.py compatibility (LDWEIGHTS is hidden by
    the PE's 64-deep reorder window when warm; no opt needed)."""
    return


def _build_nc():
    import concourse.mybir as mybir
    import concourse.tile as tile
    from concourse import bacc

    BF16 = mybir.dt.bfloat16
    F32 = mybir.dt.float32
    EXP = mybir.ActivationFunctionType.Exp
    from concourse.masks import make_identity

    nc = bacc.Bacc("TRN2", target_bir_lowering=False, debug=False,
                   num_devices=N_CORES)

    xt_ext = nc.dram_tensor("xt", [128, NTP, NDB, TP_SIZE], BF16,
                            kind="ExternalInput")
    wq_ext = nc.dram_tensor("wq", [128, G, NDB, H], BF16,
                            kind="ExternalInput")
    wk_ext = nc.dram_tensor("wk", [128, NDB, H], BF16, kind="ExternalInput")
    wv_ext = nc.dram_tensor("wv", [128, NDB, H], BF16, kind="ExternalInput")
    wo_ext = nc.dram_tensor("wo", [128, G, D], BF16, kind="ExternalInput")
    cos_ext = nc.dram_tensor("cos_t", [H, T], BF16, kind="ExternalInput")
    sin_ext = nc.dram_tensor("sin_t", [H, T], BF16, kind="ExternalInput")
    mask_ext = nc.dram_tensor("maskp", [128, 128], BF16, kind="ExternalInput")
    out_ext = nc.dram_tensor("out", [T // N_CORES, D], BF16,
                             kind="ExternalOutput")

    with tile.TileContext(nc) as tc:
        with (
            tc.tile_pool(name="consts", bufs=1) as consts,
            tc.tile_pool(name="persist", bufs=1) as persist,
            tc.tile_pool(name="xqp", bufs=8) as xqp,
            tc.tile_pool(name="qtp", bufs=2) as qtp,
            tc.tile_pool(name="csp", bufs=2) as csp,
            tc.tile_pool(name="ptp", bufs=30) as ptp,
            tc.tile_pool(name="ropep", bufs=1) as ropep,
            tc.tile_pool(name="scp", bufs=4) as scp,
            tc.tile_pool(name="osbp", bufs=2) as osbp,
            tc.tile_pool(name="qkvps", bufs=1, space="PSUM") as qkv_ps,
            tc.tile_pool(name="sps", bufs=2, space="PSUM") as sps,
            tc.tile_pool(name="smallps", bufs=3, space="PSUM") as smallps,
            tc.tile_pool(name="oprojps", bufs=2, space="PSUM") as oprojps,
            tc.tile_pool(name="dram", bufs=1, space="DRAM") as dram,
        ):
            wq_sb = consts.tile([128, G, NDB, H], BF16)
            wk_sb = consts.tile([128, NDB, H], BF16)
            wv_sb = consts.tile([128, NDB, H], BF16)
            wo_sb = consts.tile([128, G, D], BF16)
            mask_sb = consts.tile([128, 128], BF16)
            ident = consts.tile([128, 128], BF16)
            make_identity(nc, ident[:])

            kT = persist.tile([128, T], BF16)
            v_ext = persist.tile([128, NTB, VEXT_STRIDE], BF16)

            rs_in = [dram.tile([n, D], BF16, tag=f"rsw{ch}", name=f"rsw{ch}")
                     for ch, n in enumerate(CHUNK_ROWS)]
            rs_out = [dram.tile([n // N_CORES, D], BF16, tag=f"rso{ch}",
                                name=f"rso{ch}")
                      for ch, n in enumerate(CHUNK_ROWS)]

            nc.vector.memset(v_ext[:, :, 128:129], 1.0)

            # mutable emission state
            state = {
                "xq": {},      # (panel, quarter) -> sbuf tile
                "cs": {},      # panel -> (cos, sin) sbuf tiles
                "qT": {},      # (panel, g) -> roped qT tile [128, 512]
                "pts": {},     # (g, sb) -> (tile, col0) P^T tiles of cur panel
                "oT": {},      # g -> (tile, panel) of cur att panel
                "rs_done": {},
            }

            def fetch_x(p, queues=None):
                qs = queues or [nc.gpsimd] * NQ
                for q in range(NQ):
                    xq = xqp.tile([128, QDB, TP_SIZE], BF16, tag="xq",
                                  name=f"xq{p}_{q}")
                    qs[q].dma_start(
                        out=xq[:],
                        in_=xt_ext[:, p, q * QDB:(q + 1) * QDB, :])
                    state["xq"][(p, q)] = xq

            def fetch_cs(p):
                tsl = slice(p * TP_SIZE, (p + 1) * TP_SIZE)
                cos_sb = csp.tile([H, TP_SIZE], BF16, tag="cos",
                                  name=f"cos{p}")
                sin_sb = csp.tile([H, TP_SIZE], BF16, tag="sin",
                                  name=f"sin{p}")
                nc.gpsimd.dma_start(out=cos_sb[:], in_=cos_ext[:, tsl])
                nc.gpsimd.dma_start(out=sin_sb[:], in_=sin_ext[:, tsl])
                state["cs"][p] = (cos_sb, sin_sb)

            def rope(p, raw, dst):
                """dst = raw*cos + halfswap(raw)*sin for panel p [128,512]."""
                cos_sb, sin_sb = state["cs"][p]
                sw = ropep.tile([128, TP_SIZE], BF16, tag="ropesw",
                                name=f"sw{p}")
                t1 = ropep.tile([128, TP_SIZE], BF16, tag="ropet1",
                                name=f"t1{p}")
                nc.scalar.dma_start(out=sw[0:64, :], in_=raw[64:128, :])
                nc.scalar.dma_start(out=sw[64:128, :], in_=raw[0:64, :])
                nc.vector.tensor_tensor(out=t1[:], in0=raw[:], in1=cos_sb[:],
                                        op=mybir.AluOpType.mult)
                nc.vector.tensor_tensor(out=sw[:], in0=sw[:], in1=sin_sb[:],
                                        op=mybir.AluOpType.mult)
                nc.vector.tensor_tensor(out=dst[:], in0=t1[:], in1=sw[:],
                                        op=mybir.AluOpType.add)

            def qkv_unit(p, which):
                """One QKV output for panel p: 'k' | 'v' | 0..G-1."""
                ps = qkv_ps.tile([128, TP_SIZE], F32, tag="qkv",
                                 name=f"qkv{p}_{which}")
                if which == "k":
                    w = wk_sb
                elif which == "v":
                    w = wv_sb
                else:
                    w = wq_sb[:, which]
                for db in range(NDB):
                    xq = state["xq"][(p, db // QDB)]
                    nc.tensor.matmul(
                        ps[:], w[:, db, :], xq[:, db % QDB, :],
                        start=(db == 0), stop=(db == NDB - 1))
                tsl = slice(p * TP_SIZE, (p + 1) * TP_SIZE)
                if which == "k":
                    nc.scalar.copy(kT[:, tsl], ps[:])
                    rope(p, kT[:, tsl], kT[:, tsl])
                elif which == "v":
                    vraw = scp.tile([128, TP_SIZE], BF16, tag="vraw",
                                    bufs=1, name=f"vraw{p}")
                    nc.scalar.copy(vraw[:], ps[:])
                    for j in range(4):
                        sb = 4 * p + j
                        nc.sync.dma_start(
                            out=v_ext[:, sb, 0:128],
                            in_=vraw[:, j * 128:(j + 1) * 128],
                            transpose=True)
                else:
                    g = which
                    qt = qtp.tile([128, TP_SIZE], BF16, tag=f"qT{g}",
                                  name=f"qT{p}_{g}")
                    nc.vector.tensor_copy(qt[:], ps[:])
                    rope(p, qt[:], qt[:])
                    state["qT"][(p, g)] = qt

            def scores_mm(p, g, sb):
                """One score block matmul + exp + mask for (p, g, sb)."""
                qt = state["qT"][(p, g)]
                jj = sb - 4 * p
                c0 = max(jj, 0) * 128
                w = TP_SIZE - c0
                ps_s = sps.tile([128, TP_SIZE], F32, tag="s",
                                name=f"s{p}_{g}_{sb}")
                nc.tensor.matmul(
                    ps_s[:, 0:w], kT[:, sb * 128:(sb + 1) * 128],
                    qt[:, c0:TP_SIZE], start=True, stop=True)
                pt = ptp.tile([128, TP_SIZE], BF16, tag="pt",
                              name=f"pt{p}_{g}_{sb}")
                nc.scalar.activation(pt[:, 0:w], ps_s[:, 0:w], EXP,
                                     scale=SCALE)
                if jj >= 0:
                    nc.vector.tensor_tensor(
                        out=pt[:, 0:128], in0=pt[:, 0:128], in1=mask_sb[:],
                        op=mybir.AluOpType.mult)
                state["pts"][(p, g, sb)] = (pt, c0)

            def scores_unit(p, g):
                for sb in range(4 * p + 4):
                    scores_mm(p, g, sb)

            def pv_mm(p, g, j):
                """PV matmuls + DVE normalize for t-block j of head g."""
                if g not in state["oT"] or state["oT"][g][1] != p:
                    oT = scp.tile([128, TP_SIZE], BF16, tag=f"oT{g}",
                                  bufs=1, name=f"oT{p}_{g}")
                    state["oT"][g] = (oT, p)
                tb = 4 * p + j
                ps_pv = smallps.tile([128, 132], F32, tag="sm",
                                     name=f"pv{p}_{g}_{j}")
                for sb in range(tb + 1):
                    pt, c0 = state["pts"][(p, g, sb)]
                    lo = j * 128 - c0
                    nc.tensor.matmul(
                        ps_pv[:, 0:129], pt[:, lo:lo + 128],
                        v_ext[:, sb, 0:129],
                        start=(sb == 0), stop=(sb == tb),
                        skip_group_check=True)
                rc = scp.tile([128, 1], F32, tag="rc", bufs=4,
                              name=f"rc{p}_{g}_{j}")
                nc.vector.reciprocal(rc[:], ps_pv[:, 128:129])
                ob = scp.tile([128, 128], BF16, tag="ob",
                              bufs=4, name=f"ob{p}_{g}_{j}")
                nc.vector.tensor_scalar_mul(ob[:], ps_pv[:, 0:128], rc[:])
                return ob

            def pv_tr(p, g, j, ob):
                """PE transpose of normalized block + DVE copy into oT."""
                oT = state["oT"][g][0]
                ps_tr = smallps.tile([128, 128], BF16, tag="sm",
                                     name=f"tr{p}_{g}_{j}")
                nc.tensor.transpose(ps_tr[:], ob[:], ident[:])
                nc.vector.tensor_copy(oT[:, j * 128:(j + 1) * 128],
                                      ps_tr[:])

            def pv_unit(p, g, js=JSEQ):
                """Software-pipelined PV/transpose over t-blocks js."""
                obs = []
                for idx, j in enumerate(js):
                    obs.append((j, pv_mm(p, g, j)))
                    if idx >= 1:
                        jq, obq = obs.pop(0)
                        pv_tr(p, g, jq, obq)
                for jq, obq in obs:
                    pv_tr(p, g, jq, obq)

            def oproj_dq(p, j, dq, force_dve=False):
                """One quarter of o_proj for t-block j of panel p."""
                tb = 4 * p + j
                ch, idx = TB2CHUNK[tb]
                row = idx * 128
                last = p == NTP - 1
                osb = osbp.tile([128, D // 4], BF16, tag="osb",
                                name=f"osb{tb}_{dq}")
                for dp in range(2):
                    od = oprojps.tile([128, 512], F32, tag="od",
                                      name=f"od{tb}_{dq}_{dp}")
                    dc = dq * 2 + dp
                    for g in range(G):
                        nc.tensor.matmul(
                            od[:],
                            state["oT"][g][0][:, j * 128:(j + 1) * 128],
                            wo_sb[:, g, dc * 512:(dc + 1) * 512],
                            start=(g == 0), stop=(g == G - 1),
                            skip_group_check=True)
                    eng = 1 if (last or force_dve) else dq % 2
                    if eng == 0:
                        nc.scalar.copy(
                            osb[:, dp * 512:(dp + 1) * 512], od[:])
                    else:
                        nc.vector.tensor_copy(
                            osb[:, dp * 512:(dp + 1) * 512], od[:])
                q = nc.sync if (last or force_dve) else nc.scalar
                q.dma_start(
                    out=rs_in[ch][row:row + 128,
                                  dq * 1024:(dq + 1) * 1024],
                    in_=osb[:])

            def rs_maybe_trigger(tb):
                ch, _ = TB2CHUNK[tb]
                state["rs_done"].setdefault(ch, 0)
                state["rs_done"][ch] += 1
                if state["rs_done"][ch] == len(RS_CHUNKS_TB[ch]):
                    nc.gpsimd.collective_compute(
                        "ReduceScatter",
                        mybir.AluOpType.add,
                        replica_groups=[list(range(N_CORES))],
                        ins=[rs_in[ch].opt()],
                        outs=[rs_out[ch].opt()],
                    )
                    o0, o1 = CHUNK_OUT_OFF[ch], CHUNK_OUT_OFF[ch + 1]
                    nc.gpsimd.dma_start(
                        out=out_ext[o0:o1, :], in_=rs_out[ch][:])

            def oproj_unit(p, j):
                for dq in range(4):
                    oproj_dq(p, j, dq)
                rs_maybe_trigger(4 * p + j)

            # ---- block 0: QKV panel 0, DMA-arrival-ordered ----
            # Startup pieces are 256-512KB so the PE starts ~11us in.
            # Queue sequences (512KB ~ 4.3us at ~119GB/s):
            #  sync:   xq0[0]a, xq0[0]b, xq0[1], xq0[2], xq0[3],
            #          wq2_a, wq2_b, xp1[0], xp1[1]
            #  scalar: wk[0:8], wk[8:16], wk[16:32], wq0_a, wq0_b,
            #          wq3_a, wq3_b, xp1[2], xp1[3]
            #  gpsimd: mask, wv_a, wv_b, cs0, wq1_a, wq1_b, wo, cs1
            xq0 = {}
            for q in range(NQ):
                xq0[q] = xqp.tile([128, QDB, TP_SIZE], BF16, tag="xq",
                                  name=f"xq0_{q}")
                state["xq"][(0, q)] = xq0[q]
            nc.gpsimd.dma_start(out=mask_sb[:], in_=mask_ext[:])
            nc.sync.dma_start(out=xq0[0][:, 0:4, :], in_=xt_ext[:, 0, 0:4, :])
            nc.scalar.dma_start(out=wk_sb[:, 0:8], in_=wk_ext[:, 0:8])
            nc.gpsimd.dma_start(out=wv_sb[:, 0:16], in_=wv_ext[:, 0:16])
            nc.sync.dma_start(out=xq0[0][:, 4:QDB, :],
                              in_=xt_ext[:, 0, 4:QDB, :])
            nc.scalar.dma_start(out=wk_sb[:, 8:16], in_=wk_ext[:, 8:16])
            nc.gpsimd.dma_start(out=wv_sb[:, 16:32], in_=wv_ext[:, 16:32])
            nc.sync.dma_start(out=xq0[1][:], in_=xt_ext[:, 0, QDB:2 * QDB, :])
            nc.scalar.dma_start(out=wk_sb[:, 16:32], in_=wk_ext[:, 16:32])
            fetch_cs(0)
            nc.sync.dma_start(out=xq0[2][:],
                              in_=xt_ext[:, 0, 2 * QDB:3 * QDB, :])
            nc.scalar.dma_start(out=wq_sb[:, 0, 0:16], in_=wq_ext[:, 0, 0:16])
            nc.gpsimd.dma_start(out=wq_sb[:, 1, 0:16], in_=wq_ext[:, 1, 0:16])
            nc.sync.dma_start(out=xq0[3][:],
                              in_=xt_ext[:, 0, 3 * QDB:4 * QDB, :])
            nc.scalar.dma_start(out=wq_sb[:, 0, 16:32],
                                in_=wq_ext[:, 0, 16:32])
            nc.gpsimd.dma_start(out=wq_sb[:, 1, 16:32],
                                in_=wq_ext[:, 1, 16:32])
            nc.sync.dma_start(out=wq_sb[:, 2, 0:16], in_=wq_ext[:, 2, 0:16])
            nc.scalar.dma_start(out=wq_sb[:, 3, 0:16], in_=wq_ext[:, 3, 0:16])
            nc.sync.dma_start(out=wq_sb[:, 2, 16:32], in_=wq_ext[:, 2, 16:32])
            nc.scalar.dma_start(out=wq_sb[:, 3, 16:32],
                                in_=wq_ext[:, 3, 16:32])
            nc.gpsimd.dma_start(out=wo_sb[:], in_=wo_ext[:])
            # x panel 1 + cos/sin panel 1 prefetch
            fetch_x(1, queues=[nc.sync, nc.sync, nc.scalar, nc.scalar])
            fetch_cs(1)

            outs0 = ["k", "v", 0, 1, 2, 3]
            pools0 = [sps, sps, qkv_ps, sps, oprojps, oprojps]
            tags0 = ["s", "s", "qkv", "s", "od", "od"]
            ps0 = {}
            for o, pool, tg in zip(outs0, pools0, tags0):
                ps0[o] = pool.tile([128, TP_SIZE], F32, tag=tg,
                                   name=f"qkv0_{o}")
            # (output, quarter) pairs in DMA arrival order; "0a"/"0b"
            # are the two halves of quarter 0 (first k matmuls ~11us).
            PAIR_ORDER = [("k", "0a"), ("k", "0b"), ("v", 0), ("k", 1),
                          ("v", 1), (0, 0), ("k", 2), ("v", 2),
                          (1, 0), (1, 1), ("k", 3), ("v", 3),
                          (0, 1), (0, 2), (0, 3), (1, 2), (1, 3),
                          (2, 0), (2, 1), (3, 0), (3, 1),
                          (2, 2), (2, 3), (3, 2), (3, 3)]
            done = {o: 0 for o in outs0}
            for o, q in PAIR_ORDER:
                if q == "0a":
                    dbs = range(0, 4)
                elif q == "0b":
                    dbs = range(4, 8)
                else:
                    dbs = range(q * QDB, (q + 1) * QDB)
                w = (wk_sb if o == "k" else
                     wv_sb if o == "v" else wq_sb[:, o])
                for db in dbs:
                    nc.tensor.matmul(
                        ps0[o][:], w[:, db, :],
                        state["xq"][(0, db // QDB)][:, db % QDB, :],
                        start=(done[o] == 0),
                        stop=(done[o] == NDB - 1),
                        skip_group_check=True)
                    done[o] += 1
            # copy-outs, ropes, v transpose for panel 0
            nc.scalar.copy(kT[:, 0:TP_SIZE], ps0["k"][:])
            rope(0, kT[:, 0:TP_SIZE], kT[:, 0:TP_SIZE])
            vraw = scp.tile([128, TP_SIZE], BF16, tag="vraw", bufs=1,
                            name="vraw0")
            nc.scalar.copy(vraw[:], ps0["v"][:])
            for g in range(G):
                qt = qtp.tile([128, TP_SIZE], BF16, tag=f"qT{g}",
                              name=f"qT0_{g}")
                nc.scalar.copy(qt[:], ps0[g][:])
                rope(0, qt[:], qt[:])
                state["qT"][(0, g)] = qt
            for j in range(4):
                nc.sync.dma_start(out=v_ext[:, j, 0:128],
                                  in_=vraw[:, j * 128:(j + 1) * 128],
                                  transpose=True)

            # ---- blocks 1..3: att(b-1) + oproj interleaved with QKV(b) --
            for b in range(1, NTP):
                p = b - 1
                qkv = [lambda w=w, b=b: qkv_unit(b, w)
                       for w in ["k", "v", 0, 1, 2, 3]]
                att = [lambda p=p: scores_unit(p, 0)]
                for g in range(G - 1):
                    def pv_sc(g=g, p=p):
                        pv_unit(p, g)
                        scores_unit(p, g + 1)
                    att.append(pv_sc)
                # g3 + oproj tail (PV mm / PE transpose / oproj
                # staggered); for p==2 defer tb11/tb8 oproj to block 4.
                oj = JSEQ if p < 2 else JSEQ[:2]
                ts = {"obs": []}

                def pv3_push(jj, p=p, ts=ts):
                    ts["obs"].append((jj, pv_mm(p, G - 1, jj)))

                def tr_pop(p=p, ts=ts):
                    jj, ob = ts["obs"].pop(0)
                    pv_tr(p, G - 1, jj, ob)

                att.append(lambda: pv3_push(JSEQ[0]))
                att.append(lambda: (pv3_push(JSEQ[1]), tr_pop()))
                att.append(lambda p=p, oj=oj: (pv3_push(JSEQ[2]), tr_pop(),
                                               oproj_unit(p, oj[0])))
                att.append(lambda p=p, oj=oj: (pv3_push(JSEQ[3]), tr_pop(),
                                               oproj_unit(p, oj[1])))
                if len(oj) == 4:
                    att.append(lambda p=p: (tr_pop(),
                                            oproj_unit(p, JSEQ[2])))
                    att.append(lambda p=p: oproj_unit(p, JSEQ[3]))
                else:
                    att.append(lambda: tr_pop())
                n_u = max(len(att), len(qkv))
                for i in range(n_u):
                    if i < len(att):
                        att[i]()
                    if i < len(qkv):
                        qkv[i]()
                    if i == 0 and b + 1 < NTP:
                        fetch_cs(b + 1)
                    if i == 2 and b + 1 < NTP:
                        fetch_x(b + 1)

            # ---- block 4: att(panel 3) + deferred oproj(p2) ----
            # ScalarE is the bottleneck in the score phase (64 exps
            # ~41us vs 17us of score MMs), so filler PE work (deferred
            # p2 oproj quarters, then pv units) is interleaved into
            # the exp-gated score stream at ~per-3-MMs granularity.
            p = 3
            fillers = []
            for j in (JSEQ[2], JSEQ[3]):   # tb11 then tb8
                for dq in range(4):
                    fillers.append(
                        lambda j=j, dq=dq: oproj_dq(2, j, dq,
                                                    force_dve=True))
                fillers.append(lambda j=j: rs_maybe_trigger(8 + j))
            fill_i = 0
            mm_cnt = 0
            for g in range(G):
                for sb in range(4 * p + 4):
                    scores_mm(p, g, sb)
                    mm_cnt += 1
                    if mm_cnt % 3 == 0 and fill_i < len(fillers):
                        fillers[fill_i]()
                        fill_i += 1
                if g >= 1:
                    st = {"obs": []}

                    def mmf(j, g=g, st=st):
                        st["obs"].append((j, pv_mm(p, g - 1, j)))

                    def trf(g=g, st=st):
                        j, ob = st["obs"].pop(0)
                        pv_tr(p, g - 1, j, ob)

                    fillers.append(lambda f=mmf: f(JSEQ[0]))
                    fillers.append(lambda f=mmf, t=trf: (f(JSEQ[1]), t()))
                    fillers.append(lambda f=mmf, t=trf: (f(JSEQ[2]), t()))
                    fillers.append(lambda f=mmf, t=trf: (f(JSEQ[3]), t()))
                    fillers.append(lambda t=trf: t())
            while fill_i < len(fillers):
                fillers[fill_i]()
                fill_i += 1
            # tail: pv(g3) mm / tr / oproj staggered
            obs3 = []
            obs3.append((JSEQ[0], pv_mm(p, G - 1, JSEQ[0])))
            obs3.append((JSEQ[1], pv_mm(p, G - 1, JSEQ[1])))
            jj, ob = obs3.pop(0)
            pv_tr(p, G - 1, jj, ob)
            obs3.append((JSEQ[2], pv_mm(p, G - 1, JSEQ[2])))
            jj, ob = obs3.pop(0)
            pv_tr(p, G - 1, jj, ob)
            oproj_unit(p, JSEQ[0])
            obs3.append((JSEQ[3], pv_mm(p, G - 1, JSEQ[3])))
            jj, ob = obs3.pop(0)
            pv_tr(p, G - 1, jj, ob)
            oproj_unit(p, JSEQ[1])
            jj, ob = obs3.pop(0)
            pv_tr(p, G - 1, jj, ob)
            oproj_unit(p, JSEQ[2])
            oproj_unit(p, JSEQ[3])

    nc.compile()
    return nc


def get_nc():
    if "nc" not in _NC_CACHE:
        _NC_CACHE["nc"] = _build_nc()
    return _NC_CACHE["nc"]


def make_in_maps(x, positions, w_q, w_k, w_v, w_o):
    """Host-side sharding + RoPE table / mask precompute."""
    x = np.ascontiguousarray(np.asarray(x, np.float32))
    positions = np.asarray(positions)

    half = H // 2
    inv_freq = 1.0 / (THETA ** (np.arange(half, dtype=np.float32) / half))
    ang = positions.astype(np.float32)[:, None] * inv_freq[None, :]  # [T, 64]
    cos = np.cos(ang)   # [T, 64]
    sin = np.sin(ang)
    cos_t = np.empty((H, T), np.float32)
    sin_t = np.empty((H, T), np.float32)
    cos_t[0:half] = cos.T
    cos_t[half:] = cos.T
    sin_t[0:half] = -sin.T
    sin_t[half:] = sin.T
    cos_t = cos_t.astype(ml_dtypes.bfloat16)
    sin_t = sin_t.astype(ml_dtypes.bfloat16)

    # mask[s, t] = 1 if s <= t (lower-left of P^T allowed region)
    idx = np.arange(128)
    maskp = (idx[:, None] <= idx[None, :]).astype(ml_dtypes.bfloat16)

    xt = x.astype(ml_dtypes.bfloat16).T  # [D, T]
    xt4 = np.ascontiguousarray(
        xt.reshape(NDB, 128, NTP, TP_SIZE).transpose(1, 2, 0, 3))
    w_q = np.asarray(w_q, np.float32).reshape(D, NH, H).astype(
        ml_dtypes.bfloat16)
    w_k = np.asarray(w_k, np.float32).reshape(D, KH, H).astype(
        ml_dtypes.bfloat16)
    w_v = np.asarray(w_v, np.float32).reshape(D, KH, H).astype(
        ml_dtypes.bfloat16)
    w_o = np.asarray(w_o, np.float32).reshape(NH, H, D).astype(
        ml_dtypes.bfloat16)

    def blk(w):
        """[D, n] -> [128, NDB, n] with row d = a*128 + p."""
        return np.ascontiguousarray(
            w.reshape(NDB, 128, -1).transpose(1, 0, 2))

    in_maps = []
    for c in range(N_CORES):
        # wq g-major: [128, G, NDB, H] so each head's chunk is contiguous
        wq_c = w_q[:, G * c:G * (c + 1), :]            # [D, G, H]
        wq_blk = np.ascontiguousarray(
            wq_c.reshape(NDB, 128, G, H).transpose(1, 2, 0, 3))
        in_maps.append({
            "xt": xt4,
            "wq": wq_blk,
            "wk": blk(w_k[:, c, :]),
            "wv": blk(w_v[:, c, :]),
            "wo": np.ascontiguousarray(
                w_o[G * c:G * (c + 1)].reshape(G, 128, D)
                .transpose(1, 0, 2)),
            "cos_t": cos_t,
            "sin_t": sin_t,
            "maskp": maskp,
        })
    return in_maps


def assemble_output(results):
    """results: list of 8 per-core dicts with 'out' [T//8, D] bf16.

    Chunk ch rows are packed t-block-list-major; the RS gave core c
    chunk-tile rows [c*k, (c+1)*k) where k = chunk_rows/8.
    """
    out = np.empty((T, D), np.float32)
    for c in range(N_CORES):
        o = np.asarray(results[c]["out"], np.float32)
        for ch, tbs in enumerate(RS_CHUNKS_TB):
            k = CHUNK_ROWS[ch] // N_CORES
            piece = o[CHUNK_OUT_OFF[ch]:CHUNK_OUT_OFF[ch + 1]]
            for r in range(k):
                cr = c * k + r
                tb = tbs[cr // 128]
                out[tb * 128 + (cr % 128)] = piece[r]
    return out


def kernel(x, positions, w_q, w_k, w_v, w_o):
    from concourse.bass_utils import run_bass_kernel_spmd

    _enable_ldw_opt()
    nc = get_nc()
    in_maps = make_in_maps(x, positions, w_q, w_k, w_v, w_o)
    res = run_bass_kernel_spmd(nc, in_maps, core_ids=list(range(N_CORES)))
    return assemble_output(res.results)


# revision 10
# speedup vs baseline: 1.2597x; 1.0745x over previous
"""Distributed GQA attention prefill for TRN2 (8 NeuronCores).

Problem: T=2048, D=4096, N=32 query heads, K=8 kv heads, H=128.
    q = x @ w_q; k = x @ w_k; v = x @ w_v   (fused in the reference)
    rope(q), rope(k); causal GQA attention; out = o @ w_o

Sharding (tensor-parallel over heads): core c owns query heads
4c..4c+3 and kv head c (GQA groups align). w_q/w_o sharded on N,
w_k/w_v on K, x replicated. Each core computes its partial o_proj
output [T, D]; a chunked bf16 ReduceScatter sums partials; the host
concatenates the per-core row shards.

Single software-pipelined phase: the T=2048 sequence is processed in
4 panels of 512. Block b emits QKV+RoPE for panel b interleaved (in
PE program order) with attention+o_proj for panel b-1, so the PE
stream stays dense. All 80 128x128 transposes (PV outputs + V) go
through the DMA XBAR (sync queue, SBUF->SBUF) instead of the PE,
saving ~20us of PE time and 3 PSUM banks (scores get 3 rotating
banks, which un-gates the exp-paced score matmul stream).

Startup: weights/x are split into 256-512KB pieces across the 3 DMA
queues and QKV matmul (output, x-quarter) pairs are emitted in
arrival order, so the PE starts ~11us after kernel entry and never
waits on a 1MB+ transfer.

ReduceScatter: chunks are sets of 128-row t-blocks (rows are placed
into the chunk tile by completion order, not global order), sized so
the serial CC chain never backs up: 2-t-block chunks triggered at the
2nd and 4th oproj of each panel. Panel 2's last two oproj units
(tb11, tb8) are deferred into block 4 and interleaved at matmul
granularity with panel 3's exp-gated score stream (ScalarE is the
bottleneck there: 64 exps ~41us); the final chunks are 1 t-block
(tb15, tb12) so the post-PE tail is ~2 small RS ops.

Device context (measured): GPIO/SW throttle pins the PE at 13/16 =
1.95GHz for whole runs; HAM re-throttles to 1.2GHz after any >3.4us
PE gap. LDWEIGHTS is fully hidden behind matmul streaming when warm
(measured 262ns spacing for 512-wide bf16 MMs, 68ns for 129-wide),
so many small matmuls are fine. Per-queue DMA bandwidth ~119GB/s.
Uncontended CC costs: RS ~11.6us + 4.5us/MB, tiny AllReduce ~9us.
Shared-HBM dram tensors are only shared within core pairs {2c,2c+1}
(one chip), so a full manual 8-way reduction is not possible.
"""

import numpy as np
import ml_dtypes

T, D, NH, KH, H = 2048, 4096, 32, 8, 128
THETA = 10000.0
G = NH // KH          # 4 query heads per core
N_CORES = 8
TP_SIZE = 512         # t-panel
NTP = T // TP_SIZE    # 4 panels
NTB = T // 128        # 16 t/s blocks
NDB = D // 128        # 32 d blocks
NQ = 4                # x quarters per panel (8 d-blocks each)
QDB = NDB // NQ
SCALE = 1.0 / float(np.sqrt(H))
VEXT_STRIDE = 160     # v_ext row stride (129 used; 320B so XBAR
                      # transpose dsts stay 64B-aligned)
JSEQ = [1, 2, 3, 0]   # t-block order within a panel

# ReduceScatter chunks: lists of t-blocks, in completion order given
# JSEQ. Chunk rows are packed in list order (idx*128), not global row
# order; assemble_output unpacks. Panel p completes tb 4p+1, 4p+2,
# 4p+3, 4p+0; panel 2's last two (tb11, tb8) are deferred to block 4.
RS_CHUNKS_TB = [[1, 2], [3, 0], [5, 6], [7, 4], [9, 10], [11, 8],
                [13, 14], [15], [12]]
TB2CHUNK = {tb: (ch, idx) for ch, tbs in enumerate(RS_CHUNKS_TB)
            for idx, tb in enumerate(tbs)}
CHUNK_ROWS = [128 * len(tbs) for tbs in RS_CHUNKS_TB]
CHUNK_OUT_OFF = np.concatenate(
    [[0], np.cumsum([n // N_CORES for n in CHUNK_ROWS])]).tolist()

_NC_CACHE = {}


def _enable_ldw_opt():
    """No-op kept for test.py compatibility (LDWEIGHTS is hidden by
    the PE's 64-deep reorder window when warm; no opt needed)."""
    return


def _build_nc():
    import concourse.mybir as mybir
    import concourse.tile as tile
    from concourse import bacc

    BF16 = mybir.dt.bfloat16
    F32 = mybir.dt.float32
    EXP = mybir.ActivationFunctionType.Exp
    from concourse.masks import make_identity

    nc = bacc.Bacc("TRN2", target_bir_lowering=False, debug=False,
                   num_devices=N_CORES)

    xt_ext = nc.dram_tensor("xt", [128, NTP, NDB, TP_SIZE], BF16,
                            kind="ExternalInput")
    wq_ext = nc.dram_tensor("wq", [128, G, NDB, H], BF16,
                            kind="ExternalInput")
    wk_ext = nc.dram_tensor("wk", [128, NDB, H], BF16, kind="ExternalInput")
    wv_ext = nc.dram_tensor("wv", [128, NDB, H], BF16, kind="ExternalInput")
    wo_ext = nc.dram_tensor("wo", [128, G, D], BF16, kind="ExternalInput")
    cos_ext = nc.dram_tensor("cos_t", [H, T], BF16, kind="ExternalInput")
    sin_ext = nc.dram_tensor("sin_t", [H, T], BF16, kind="ExternalInput")
    mask_ext = nc.dram_tensor("maskp", [128, 128], BF16, kind="ExternalInput")
    out_ext = nc.dram_tensor("out", [T // N_CORES, D], BF16,
                             kind="ExternalOutput")

    with tile.TileContext(nc) as tc:
        with (
            tc.tile_pool(name="consts", bufs=1) as consts,
            tc.tile_pool(name="persist", bufs=1) as persist,
            tc.tile_pool(name="xqp", bufs=8) as xqp,
            tc.tile_pool(name="qtp", bufs=2) as qtp,
            tc.tile_pool(name="csp", bufs=2) as csp,
            tc.tile_pool(name="ptp", bufs=28) as ptp,
            tc.tile_pool(name="ropep", bufs=1) as ropep,
            tc.tile_pool(name="scp", bufs=4) as scp,
            tc.tile_pool(name="osbp", bufs=3) as osbp,
            tc.tile_pool(name="qkvps", bufs=1, space="PSUM") as qkv_ps,
            tc.tile_pool(name="sps", bufs=2, space="PSUM") as sps,
            tc.tile_pool(name="smallps", bufs=3, space="PSUM") as smallps,
            tc.tile_pool(name="oprojps", bufs=2, space="PSUM") as oprojps,
            tc.tile_pool(name="dram", bufs=1, space="DRAM") as dram,
        ):
            wq_sb = consts.tile([128, G, NDB, H], BF16)
            wk_sb = consts.tile([128, NDB, H], BF16)
            wv_sb = consts.tile([128, NDB, H], BF16)
            wo_sb = consts.tile([128, G, D], BF16)
            mask_sb = consts.tile([128, 128], BF16)
            ident = consts.tile([128, 128], BF16)
            make_identity(nc, ident[:])

            kT = persist.tile([128, T], BF16)
            v_ext = persist.tile([128, NTB, VEXT_STRIDE], BF16)

            rs_in = [dram.tile([n, D], BF16, tag=f"rsw{ch}", name=f"rsw{ch}")
                     for ch, n in enumerate(CHUNK_ROWS)]
            rs_out = [dram.tile([n // N_CORES, D], BF16, tag=f"rso{ch}",
                                name=f"rso{ch}")
                      for ch, n in enumerate(CHUNK_ROWS)]

            nc.vector.memset(v_ext[:, :, 128:129], 1.0)

            # mutable emission state
            state = {
                "xq": {},      # (panel, quarter) -> sbuf tile
                "cs": {},      # panel -> (cos, sin) sbuf tiles
                "qT": {},      # (panel, g) -> roped qT tile [128, 512]
                "pts": {},     # (g, sb) -> (tile, col0) P^T tiles of cur panel
                "oT": {},      # g -> (tile, panel) of cur att panel
                "rs_done": {},
            }

            def fetch_x(p, queues=None):
                qs = queues or [nc.gpsimd] * NQ
                for q in range(NQ):
                    xq = xqp.tile([128, QDB, TP_SIZE], BF16, tag="xq",
                                  name=f"xq{p}_{q}")
                    qs[q].dma_start(
                        out=xq[:],
                        in_=xt_ext[:, p, q * QDB:(q + 1) * QDB, :])
                    state["xq"][(p, q)] = xq

            def fetch_cs(p):
                tsl = slice(p * TP_SIZE, (p + 1) * TP_SIZE)
                cos_sb = csp.tile([H, TP_SIZE], BF16, tag="cos",
                                  name=f"cos{p}")
                sin_sb = csp.tile([H, TP_SIZE], BF16, tag="sin",
                                  name=f"sin{p}")
                nc.gpsimd.dma_start(out=cos_sb[:], in_=cos_ext[:, tsl])
                nc.gpsimd.dma_start(out=sin_sb[:], in_=sin_ext[:, tsl])
                state["cs"][p] = (cos_sb, sin_sb)

            def rope(p, raw, dst):
                """dst = raw*cos + halfswap(raw)*sin for panel p [128,512]."""
                cos_sb, sin_sb = state["cs"][p]
                sw = ropep.tile([128, TP_SIZE], BF16, tag="ropesw",
                                name=f"sw{p}")
                t1 = ropep.tile([128, TP_SIZE], BF16, tag="ropet1",
                                name=f"t1{p}")
                nc.scalar.dma_start(out=sw[0:64, :], in_=raw[64:128, :])
                nc.scalar.dma_start(out=sw[64:128, :], in_=raw[0:64, :])
                nc.vector.tensor_tensor(out=t1[:], in0=raw[:], in1=cos_sb[:],
                                        op=mybir.AluOpType.mult)
                nc.vector.tensor_tensor(out=sw[:], in0=sw[:], in1=sin_sb[:],
                                        op=mybir.AluOpType.mult)
                nc.vector.tensor_tensor(out=dst[:], in0=t1[:], in1=sw[:],
                                        op=mybir.AluOpType.add)

            def qkv_unit(p, which):
                """One QKV output for panel p: 'k' | 'v' | 0..G-1."""
                ps = qkv_ps.tile([128, TP_SIZE], F32, tag="qkv",
                                 name=f"qkv{p}_{which}")
                if which == "k":
                    w = wk_sb
                elif which == "v":
                    w = wv_sb
                else:
                    w = wq_sb[:, which]
                for db in range(NDB):
                    xq = state["xq"][(p, db // QDB)]
                    nc.tensor.matmul(
                        ps[:], w[:, db, :], xq[:, db % QDB, :],
                        start=(db == 0), stop=(db == NDB - 1))
                tsl = slice(p * TP_SIZE, (p + 1) * TP_SIZE)
                if which == "k":
                    nc.scalar.copy(kT[:, tsl], ps[:])
                    rope(p, kT[:, tsl], kT[:, tsl])
                elif which == "v":
                    vraw = scp.tile([128, TP_SIZE], BF16, tag="vraw",
                                    bufs=1, name=f"vraw{p}")
                    nc.scalar.copy(vraw[:], ps[:])
                    for j in range(4):
                        sb = 4 * p + j
                        nc.sync.dma_start(
                            out=v_ext[:, sb, 0:128],
                            in_=vraw[:, j * 128:(j + 1) * 128],
                            transpose=True)
                else:
                    g = which
                    qt = qtp.tile([128, TP_SIZE], BF16, tag=f"qT{g}",
                                  name=f"qT{p}_{g}")
                    nc.vector.tensor_copy(qt[:], ps[:])
                    rope(p, qt[:], qt[:])
                    state["qT"][(p, g)] = qt

            def scores_mm(p, g, sb):
                """One score block matmul + exp + mask for (p, g, sb)."""
                qt = state["qT"][(p, g)]
                jj = sb - 4 * p
                c0 = max(jj, 0) * 128
                w = TP_SIZE - c0
                ps_s = sps.tile([128, TP_SIZE], F32, tag="s",
                                name=f"s{p}_{g}_{sb}")
                nc.tensor.matmul(
                    ps_s[:, 0:w], kT[:, sb * 128:(sb + 1) * 128],
                    qt[:, c0:TP_SIZE], start=True, stop=True)
                pt = ptp.tile([128, TP_SIZE], BF16, tag="pt",
                              name=f"pt{p}_{g}_{sb}")
                nc.scalar.activation(pt[:, 0:w], ps_s[:, 0:w], EXP,
                                     scale=SCALE)
                if jj >= 0:
                    nc.vector.tensor_tensor(
                        out=pt[:, 0:128], in0=pt[:, 0:128], in1=mask_sb[:],
                        op=mybir.AluOpType.mult)
                state["pts"][(p, g, sb)] = (pt, c0)

            def scores_unit(p, g):
                for sb in range(4 * p + 4):
                    scores_mm(p, g, sb)

            def pv_mm(p, g, j):
                """PV matmuls + DVE normalize for t-block j of head g."""
                if g not in state["oT"] or state["oT"][g][1] != p:
                    oT = scp.tile([128, TP_SIZE], BF16, tag=f"oT{g}",
                                  bufs=1, name=f"oT{p}_{g}")
                    state["oT"][g] = (oT, p)
                tb = 4 * p + j
                ps_pv = smallps.tile([128, 132], F32, tag="sm",
                                     name=f"pv{p}_{g}_{j}")
                for sb in range(tb + 1):
                    pt, c0 = state["pts"][(p, g, sb)]
                    lo = j * 128 - c0
                    nc.tensor.matmul(
                        ps_pv[:, 0:129], pt[:, lo:lo + 128],
                        v_ext[:, sb, 0:129],
                        start=(sb == 0), stop=(sb == tb),
                        skip_group_check=True)
                rc = scp.tile([128, 1], F32, tag="rc", bufs=4,
                              name=f"rc{p}_{g}_{j}")
                nc.vector.reciprocal(rc[:], ps_pv[:, 128:129])
                ob = scp.tile([128, 128], BF16, tag="ob",
                              bufs=4, name=f"ob{p}_{g}_{j}")
                nc.vector.tensor_scalar_mul(ob[:], ps_pv[:, 0:128], rc[:])
                return ob

            def pv_tr(p, g, j, ob):
                """PE transpose of normalized block + DVE copy into oT."""
                oT = state["oT"][g][0]
                ps_tr = smallps.tile([128, 128], BF16, tag="sm",
                                     name=f"tr{p}_{g}_{j}")
                nc.tensor.transpose(ps_tr[:], ob[:], ident[:])
                nc.vector.tensor_copy(oT[:, j * 128:(j + 1) * 128],
                                      ps_tr[:])

            def pv_unit(p, g, js=JSEQ):
                """Software-pipelined PV/transpose over t-blocks js."""
                obs = []
                for idx, j in enumerate(js):
                    obs.append((j, pv_mm(p, g, j)))
                    if idx >= 1:
                        jq, obq = obs.pop(0)
                        pv_tr(p, g, jq, obq)
                for jq, obq in obs:
                    pv_tr(p, g, jq, obq)

            def oproj_dq(p, j, dq, force_dve=False):
                """One quarter of o_proj for t-block j of panel p."""
                tb = 4 * p + j
                ch, idx = TB2CHUNK[tb]
                row = idx * 128
                last = p == NTP - 1
                osb = osbp.tile([128, D // 4], BF16, tag="osb",
                                name=f"osb{tb}_{dq}")
                for dp in range(2):
                    od = oprojps.tile([128, 512], F32, tag="od",
                                      name=f"od{tb}_{dq}_{dp}")
                    dc = dq * 2 + dp
                    for g in range(G):
                        nc.tensor.matmul(
                            od[:],
                            state["oT"][g][0][:, j * 128:(j + 1) * 128],
                            wo_sb[:, g, dc * 512:(dc + 1) * 512],
                            start=(g == 0), stop=(g == G - 1),
                            skip_group_check=True)
                    eng = 1 if (last or force_dve) else dq % 2
                    if eng == 0:
                        nc.scalar.copy(
                            osb[:, dp * 512:(dp + 1) * 512], od[:])
                    else:
                        nc.vector.tensor_copy(
                            osb[:, dp * 512:(dp + 1) * 512], od[:])
                q = nc.sync if (last or force_dve or dq % 2 == 1) \
                    else nc.scalar
                q.dma_start(
                    out=rs_in[ch][row:row + 128,
                                  dq * 1024:(dq + 1) * 1024],
                    in_=osb[:])

            def rs_maybe_trigger(tb):
                ch, _ = TB2CHUNK[tb]
                state["rs_done"].setdefault(ch, 0)
                state["rs_done"][ch] += 1
                if state["rs_done"][ch] == len(RS_CHUNKS_TB[ch]):
                    nc.gpsimd.collective_compute(
                        "ReduceScatter",
                        mybir.AluOpType.add,
                        replica_groups=[list(range(N_CORES))],
                        ins=[rs_in[ch].opt()],
                        outs=[rs_out[ch].opt()],
                    )
                    o0, o1 = CHUNK_OUT_OFF[ch], CHUNK_OUT_OFF[ch + 1]
                    nc.gpsimd.dma_start(
                        out=out_ext[o0:o1, :], in_=rs_out[ch][:])

            def oproj_unit(p, j):
                for dq in range(4):
                    oproj_dq(p, j, dq)
                rs_maybe_trigger(4 * p + j)

            # ---- block 0: QKV panel 0, DMA-arrival-ordered ----
            # Startup pieces are 256-512KB so the PE starts ~11us in.
            # Queue sequences (512KB ~ 4.3us at ~119GB/s):
            #  sync:   xq0[0]a, xq0[0]b, xq0[1], xq0[2], xq0[3],
            #          wq2_a, wq2_b, xp1[0], xp1[1]
            #  scalar: wk[0:8], wk[8:16], wk[16:32], wq0_a, wq0_b,
            #          wq3_a, wq3_b, xp1[2], xp1[3]
            #  gpsimd: mask, wv_a, wv_b, cs0, wq1_a, wq1_b, wo, cs1
            xq0 = {}
            for q in range(NQ):
                xq0[q] = xqp.tile([128, QDB, TP_SIZE], BF16, tag="xq",
                                  name=f"xq0_{q}")
                state["xq"][(0, q)] = xq0[q]
            nc.gpsimd.dma_start(out=mask_sb[:], in_=mask_ext[:])
            nc.sync.dma_start(out=xq0[0][:, 0:4, :], in_=xt_ext[:, 0, 0:4, :])
            nc.scalar.dma_start(out=wk_sb[:, 0:8], in_=wk_ext[:, 0:8])
            nc.gpsimd.dma_start(out=wv_sb[:, 0:16], in_=wv_ext[:, 0:16])
            nc.sync.dma_start(out=xq0[0][:, 4:QDB, :],
                              in_=xt_ext[:, 0, 4:QDB, :])
            nc.scalar.dma_start(out=wk_sb[:, 8:16], in_=wk_ext[:, 8:16])
            nc.gpsimd.dma_start(out=wv_sb[:, 16:32], in_=wv_ext[:, 16:32])
            nc.sync.dma_start(out=xq0[1][:], in_=xt_ext[:, 0, QDB:2 * QDB, :])
            nc.scalar.dma_start(out=wk_sb[:, 16:32], in_=wk_ext[:, 16:32])
            fetch_cs(0)
            nc.sync.dma_start(out=xq0[2][:],
                              in_=xt_ext[:, 0, 2 * QDB:3 * QDB, :])
            nc.scalar.dma_start(out=wq_sb[:, 0, 0:16], in_=wq_ext[:, 0, 0:16])
            nc.gpsimd.dma_start(out=wq_sb[:, 1, 0:16], in_=wq_ext[:, 1, 0:16])
            nc.sync.dma_start(out=xq0[3][:],
                              in_=xt_ext[:, 0, 3 * QDB:4 * QDB, :])
            nc.scalar.dma_start(out=wq_sb[:, 0, 16:32],
                                in_=wq_ext[:, 0, 16:32])
            nc.gpsimd.dma_start(out=wq_sb[:, 1, 16:32],
                                in_=wq_ext[:, 1, 16:32])
            nc.sync.dma_start(out=wq_sb[:, 2, 0:16], in_=wq_ext[:, 2, 0:16])
            nc.scalar.dma_start(out=wq_sb[:, 3, 0:16], in_=wq_ext[:, 3, 0:16])
            nc.sync.dma_start(out=wq_sb[:, 2, 16:32], in_=wq_ext[:, 2, 16:32])
            nc.scalar.dma_start(out=wq_sb[:, 3, 16:32],
                                in_=wq_ext[:, 3, 16:32])
            nc.gpsimd.dma_start(out=wo_sb[:], in_=wo_ext[:])

            outs0 = ["k", "v", 0, 1, 2, 3]
            pools0 = [sps, sps, qkv_ps, sps, oprojps, oprojps]
            tags0 = ["s", "s", "qkv", "s", "od", "od"]
            ps0 = {}
            for o, pool, tg in zip(outs0, pools0, tags0):
                ps0[o] = pool.tile([128, TP_SIZE], F32, tag=tg,
                                   name=f"qkv0_{o}")
            # (output, quarter) pairs in DMA arrival order; "0a"/"0b"
            # are the two halves of quarter 0 (first k matmuls ~11us).
            PAIR_ORDER = [("k", "0a"), ("k", "0b"), ("v", 0), ("k", 1),
                          ("v", 1), (0, 0), ("k", 2), ("v", 2),
                          (1, 0), (1, 1), ("k", 3), ("v", 3),
                          (0, 1), (0, 2), (0, 3), (1, 2), (1, 3),
                          (2, 0), (2, 1), (3, 0), (3, 1),
                          (2, 2), (2, 3), (3, 2), (3, 3)]
            done = {o: 0 for o in outs0}

            def copyout0(o):
                """Copy-out + rope for output o as soon as it closes."""
                if o == "k":
                    nc.scalar.copy(kT[:, 0:TP_SIZE], ps0["k"][:])
                    rope(0, kT[:, 0:TP_SIZE], kT[:, 0:TP_SIZE])
                elif o == "v":
                    vraw = scp.tile([128, TP_SIZE], BF16, tag="vraw",
                                    bufs=1, name="vraw0")
                    nc.scalar.copy(vraw[:], ps0["v"][:])
                    for j in range(4):
                        nc.sync.dma_start(
                            out=v_ext[:, j, 0:128],
                            in_=vraw[:, j * 128:(j + 1) * 128],
                            transpose=True)
                else:
                    qt = qtp.tile([128, TP_SIZE], BF16, tag=f"qT{o}",
                                  name=f"qT0_{o}")
                    nc.scalar.copy(qt[:], ps0[o][:])
                    rope(0, qt[:], qt[:])
                    state["qT"][(0, o)] = qt

            for o, q in PAIR_ORDER:
                if q == "0a":
                    dbs = range(0, 4)
                elif q == "0b":
                    dbs = range(4, 8)
                else:
                    dbs = range(q * QDB, (q + 1) * QDB)
                w = (wk_sb if o == "k" else
                     wv_sb if o == "v" else wq_sb[:, o])
                for db in dbs:
                    nc.tensor.matmul(
                        ps0[o][:], w[:, db, :],
                        state["xq"][(0, db // QDB)][:, db % QDB, :],
                        start=(done[o] == 0),
                        stop=(done[o] == NDB - 1),
                        skip_group_check=True)
                    done[o] += 1
                if done[o] == NDB:
                    copyout0(o)
            # x panel 1 + cos/sin prefetch after the rope/copy DMAs so
            # the small latency-critical transfers run first.
            fetch_x(1, queues=[nc.sync, nc.sync, nc.scalar, nc.scalar])
            fetch_cs(1)

            # ---- blocks 1..3: att(b-1) + oproj interleaved with QKV(b) --
            for b in range(1, NTP):
                p = b - 1
                qkv = [lambda w=w, b=b: qkv_unit(b, w)
                       for w in ["k", "v", 0, 1, 2, 3]]
                att = [lambda p=p: scores_unit(p, 0)]
                for g in range(G - 1):
                    def pv_sc(g=g, p=p):
                        pv_unit(p, g)
                        scores_unit(p, g + 1)
                    att.append(pv_sc)
                # g3 + oproj tail (PV mm / PE transpose / oproj
                # staggered); for p==2 defer tb11/tb8 oproj to block 4.
                oj = JSEQ if p < 2 else JSEQ[:2]
                ts = {"obs": []}

                def pv3_push(jj, p=p, ts=ts):
                    ts["obs"].append((jj, pv_mm(p, G - 1, jj)))

                def tr_pop(p=p, ts=ts):
                    jj, ob = ts["obs"].pop(0)
                    pv_tr(p, G - 1, jj, ob)

                att.append(lambda: pv3_push(JSEQ[0]))
                att.append(lambda: (pv3_push(JSEQ[1]), tr_pop()))
                att.append(lambda p=p, oj=oj: (pv3_push(JSEQ[2]), tr_pop(),
                                               oproj_unit(p, oj[0])))
                att.append(lambda p=p, oj=oj: (pv3_push(JSEQ[3]), tr_pop(),
                                               oproj_unit(p, oj[1])))
                if len(oj) == 4:
                    att.append(lambda p=p: (tr_pop(),
                                            oproj_unit(p, JSEQ[2])))
                    att.append(lambda p=p: oproj_unit(p, JSEQ[3]))
                else:
                    att.append(lambda: tr_pop())
                n_u = max(len(att), len(qkv))
                for i in range(n_u):
                    if i < len(att):
                        att[i]()
                    if i < len(qkv):
                        qkv[i]()
                    if i == 0 and b + 1 < NTP:
                        fetch_cs(b + 1)
                    if i == 2 and b + 1 < NTP:
                        fetch_x(b + 1)

            # ---- block 4: att(panel 3) + deferred oproj(p2) ----
            # ScalarE is the bottleneck in the score phase (64 exps
            # ~41us vs 17us of score MMs), so filler PE work (deferred
            # p2 oproj quarters, then pv units) is interleaved into
            # the exp-gated score stream at ~per-3-MMs granularity.
            p = 3
            fillers = []
            for j in (JSEQ[2], JSEQ[3]):   # tb11 then tb8
                for dq in range(4):
                    fillers.append(
                        lambda j=j, dq=dq: oproj_dq(2, j, dq,
                                                    force_dve=True))
                fillers.append(lambda j=j: rs_maybe_trigger(8 + j))
            fill_i = 0
            mm_cnt = 0
            for g in range(G):
                for sb in range(4 * p + 4):
                    scores_mm(p, g, sb)
                    mm_cnt += 1
                    if mm_cnt % 3 == 0 and fill_i < len(fillers):
                        fillers[fill_i]()
                        fill_i += 1
                if g >= 1:
                    st = {"obs": []}

                    def mmf(j, g=g, st=st):
                        st["obs"].append((j, pv_mm(p, g - 1, j)))

                    def trf(g=g, st=st):
                        j, ob = st["obs"].pop(0)
                        pv_tr(p, g - 1, j, ob)

                    fillers.append(lambda f=mmf: f(JSEQ[0]))
                    fillers.append(lambda f=mmf, t=trf: (f(JSEQ[1]), t()))
                    fillers.append(lambda f=mmf, t=trf: (f(JSEQ[2]), t()))
                    fillers.append(lambda f=mmf, t=trf: (f(JSEQ[3]), t()))
                    fillers.append(lambda t=trf: t())
            while fill_i < len(fillers):
                fillers[fill_i]()
                fill_i += 1
            # tail: pv(g3) mm / tr / oproj staggered
            obs3 = []
            obs3.append((JSEQ[0], pv_mm(p, G - 1, JSEQ[0])))
            obs3.append((JSEQ[1], pv_mm(p, G - 1, JSEQ[1])))
            jj, ob = obs3.pop(0)
            pv_tr(p, G - 1, jj, ob)
            obs3.append((JSEQ[2], pv_mm(p, G - 1, JSEQ[2])))
            jj, ob = obs3.pop(0)
            pv_tr(p, G - 1, jj, ob)
            oproj_unit(p, JSEQ[0])
            obs3.append((JSEQ[3], pv_mm(p, G - 1, JSEQ[3])))
            jj, ob = obs3.pop(0)
            pv_tr(p, G - 1, jj, ob)
            oproj_unit(p, JSEQ[1])
            jj, ob = obs3.pop(0)
            pv_tr(p, G - 1, jj, ob)
            oproj_unit(p, JSEQ[2])
            oproj_unit(p, JSEQ[3])

    nc.compile()
    return nc


def get_nc():
    if "nc" not in _NC_CACHE:
        _NC_CACHE["nc"] = _build_nc()
    return _NC_CACHE["nc"]


def make_in_maps(x, positions, w_q, w_k, w_v, w_o):
    """Host-side sharding + RoPE table / mask precompute."""
    x = np.ascontiguousarray(np.asarray(x, np.float32))
    positions = np.asarray(positions)

    half = H // 2
    inv_freq = 1.0 / (THETA ** (np.arange(half, dtype=np.float32) / half))
    ang = positions.astype(np.float32)[:, None] * inv_freq[None, :]  # [T, 64]
    cos = np.cos(ang)   # [T, 64]
    sin = np.sin(ang)
    cos_t = np.empty((H, T), np.float32)
    sin_t = np.empty((H, T), np.float32)
    cos_t[0:half] = cos.T
    cos_t[half:] = cos.T
    sin_t[0:half] = -sin.T
    sin_t[half:] = sin.T
    cos_t = cos_t.astype(ml_dtypes.bfloat16)
    sin_t = sin_t.astype(ml_dtypes.bfloat16)

    # mask[s, t] = 1 if s <= t (lower-left of P^T allowed region)
    idx = np.arange(128)
    maskp = (idx[:, None] <= idx[None, :]).astype(ml_dtypes.bfloat16)

    xt = x.astype(ml_dtypes.bfloat16).T  # [D, T]
    xt4 = np.ascontiguousarray(
        xt.reshape(NDB, 128, NTP, TP_SIZE).transpose(1, 2, 0, 3))
    w_q = np.asarray(w_q, np.float32).reshape(D, NH, H).astype(
        ml_dtypes.bfloat16)
    w_k = np.asarray(w_k, np.float32).reshape(D, KH, H).astype(
        ml_dtypes.bfloat16)
    w_v = np.asarray(w_v, np.float32).reshape(D, KH, H).astype(
        ml_dtypes.bfloat16)
    w_o = np.asarray(w_o, np.float32).reshape(NH, H, D).astype(
        ml_dtypes.bfloat16)

    def blk(w):
        """[D, n] -> [128, NDB, n] with row d = a*128 + p."""
        return np.ascontiguousarray(
            w.reshape(NDB, 128, -1).transpose(1, 0, 2))

    in_maps = []
    for c in range(N_CORES):
        # wq g-major: [128, G, NDB, H] so each head's chunk is contiguous
        wq_c = w_q[:, G * c:G * (c + 1), :]            # [D, G, H]
        wq_blk = np.ascontiguousarray(
            wq_c.reshape(NDB, 128, G, H).transpose(1, 2, 0, 3))
        in_maps.append({
            "xt": xt4,
            "wq": wq_blk,
            "wk": blk(w_k[:, c, :]),
            "wv": blk(w_v[:, c, :]),
            "wo": np.ascontiguousarray(
                w_o[G * c:G * (c + 1)].reshape(G, 128, D)
                .transpose(1, 0, 2)),
            "cos_t": cos_t,
            "sin_t": sin_t,
            "maskp": maskp,
        })
    return in_maps


def assemble_output(results):
    """results: list of 8 per-core dicts with 'out' [T//8, D] bf16.

    Chunk ch rows are packed t-block-list-major; the RS gave core c
    chunk-tile rows [c*k, (c+1)*k) where k = chunk_rows/8.
    """
    out = np.empty((T, D), np.float32)
    for c in range(N_CORES):
        o = np.asarray(results[c]["out"], np.float32)
        for ch, tbs in enumerate(RS_CHUNKS_TB):
            k = CHUNK_ROWS[ch] // N_CORES
            piece = o[CHUNK_OUT_OFF[ch]:CHUNK_OUT_OFF[ch + 1]]
            for r in range(k):
                cr = c * k + r
                tb = tbs[cr // 128]
                out[tb * 128 + (cr % 128)] = piece[r]
    return out


def kernel(x, positions, w_q, w_k, w_v, w_o):
    from concourse.bass_utils import run_bass_kernel_spmd

    _enable_ldw_opt()
    nc = get_nc()
    in_maps = make_in_maps(x, positions, w_q, w_k, w_v, w_o)
    res = run_bass_kernel_spmd(nc, in_maps, core_ids=list(range(N_CORES)))
    return assemble_output(res.results)


# revision 12
# speedup vs baseline: 1.2742x; 1.0115x over previous
"""Distributed GQA attention prefill for TRN2 (8 NeuronCores).

Problem: T=2048, D=4096, N=32 query heads, K=8 kv heads, H=128.
    q = x @ w_q; k = x @ w_k; v = x @ w_v   (fused in the reference)
    rope(q), rope(k); causal GQA attention; out = o @ w_o

Sharding (tensor-parallel over heads): core c owns query heads
4c..4c+3 and kv head c (GQA groups align). w_q/w_o sharded on N,
w_k/w_v on K, x replicated. Each core computes its partial o_proj
output [T, D]; a chunked bf16 ReduceScatter sums partials; the host
concatenates the per-core row shards.

Single software-pipelined phase: the T=2048 sequence is processed in
4 panels of 512. Block b emits QKV+RoPE for panel b interleaved (in
PE program order) with attention+o_proj for panel b-1, so the PE
stream stays dense. All 80 128x128 transposes (PV outputs + V) go
through the DMA XBAR (sync queue, SBUF->SBUF) instead of the PE,
saving ~20us of PE time and 3 PSUM banks (scores get 3 rotating
banks, which un-gates the exp-paced score matmul stream).

Startup: weights/x are split into 256-512KB pieces across the 3 DMA
queues and QKV matmul (output, x-quarter) pairs are emitted in
arrival order, so the PE starts ~11us after kernel entry and never
waits on a 1MB+ transfer.

ReduceScatter: chunks are sets of 128-row t-blocks (rows are placed
into the chunk tile by completion order, not global order), sized so
the serial CC chain never backs up: 2-t-block chunks triggered at the
2nd and 4th oproj of each panel. Panel 2's last two oproj units
(tb11, tb8) are deferred into block 4 and interleaved at matmul
granularity with panel 3's exp-gated score stream (ScalarE is the
bottleneck there: 64 exps ~41us); the final chunks are 1 t-block
(tb15, tb12) so the post-PE tail is ~2 small RS ops.

Device context (measured): GPIO/SW throttle pins the PE at 13/16 =
1.95GHz for whole runs; HAM re-throttles to 1.2GHz after any >3.4us
PE gap. LDWEIGHTS is fully hidden behind matmul streaming when warm
(measured 262ns spacing for 512-wide bf16 MMs, 68ns for 129-wide),
so many small matmuls are fine. Per-queue DMA bandwidth ~119GB/s.
Uncontended CC costs: RS ~11.6us + 4.5us/MB, tiny AllReduce ~9us.
Shared-HBM dram tensors are only shared within core pairs {2c,2c+1}
(one chip), so a full manual 8-way reduction is not possible.
"""

import numpy as np
import ml_dtypes

T, D, NH, KH, H = 2048, 4096, 32, 8, 128
THETA = 10000.0
G = NH // KH          # 4 query heads per core
N_CORES = 8
TP_SIZE = 512         # t-panel
NTP = T // TP_SIZE    # 4 panels
NTB = T // 128        # 16 t/s blocks
NDB = D // 128        # 32 d blocks
NQ = 4                # x quarters per panel (8 d-blocks each)
QDB = NDB // NQ
SCALE = 1.0 / float(np.sqrt(H))
VEXT_STRIDE = 160     # v_ext row stride (129 used; 320B so XBAR
                      # transpose dsts stay 64B-aligned)
JSEQ = [1, 2, 3, 0]   # t-block order within a panel

# ReduceScatter chunks: lists of t-blocks, in completion order given
# JSEQ. Chunk rows are packed in list order (idx*128), not global row
# order; assemble_output unpacks. Panel p completes tb 4p+1, 4p+2,
# 4p+3, 4p+0; panel 2's last two (tb11, tb8) are deferred to block 4.
RS_CHUNKS_TB = [[1, 2], [3, 0], [5, 6], [7, 4], [9, 10], [11, 8],
                [13, 14], [15, 12]]
TB2CHUNK = {tb: (ch, idx) for ch, tbs in enumerate(RS_CHUNKS_TB)
            for idx, tb in enumerate(tbs)}
CHUNK_ROWS = [128 * len(tbs) for tbs in RS_CHUNKS_TB]
CHUNK_OUT_OFF = np.concatenate(
    [[0], np.cumsum([n // N_CORES for n in CHUNK_ROWS])]).tolist()

_NC_CACHE = {}


def _enable_ldw_opt():
    """No-op kept for test.py compatibility (LDWEIGHTS is hidden by
    the PE's 64-deep reorder window when warm; no opt needed)."""
    return


def _build_nc():
    import concourse.mybir as mybir
    import concourse.tile as tile
    from concourse import bacc

    BF16 = mybir.dt.bfloat16
    F32 = mybir.dt.float32
    EXP = mybir.ActivationFunctionType.Exp
    from concourse.masks import make_identity

    nc = bacc.Bacc("TRN2", target_bir_lowering=False, debug=False,
                   num_devices=N_CORES)

    xt_ext = nc.dram_tensor("xt", [128, NTP, NDB, TP_SIZE], BF16,
                            kind="ExternalInput")
    wq_ext = nc.dram_tensor("wq", [128, G, NDB, H], BF16,
                            kind="ExternalInput")
    wk_ext = nc.dram_tensor("wk", [128, NDB, H], BF16, kind="ExternalInput")
    wv_ext = nc.dram_tensor("wv", [128, NDB, H], BF16, kind="ExternalInput")
    wo_ext = nc.dram_tensor("wo", [128, G, D], BF16, kind="ExternalInput")
    cos_ext = nc.dram_tensor("cos_t", [H, T], BF16, kind="ExternalInput")
    sin_ext = nc.dram_tensor("sin_t", [H, T], BF16, kind="ExternalInput")
    mask_ext = nc.dram_tensor("maskp", [128, 128], BF16, kind="ExternalInput")
    out_ext = nc.dram_tensor("out", [T // N_CORES, D], BF16,
                             kind="ExternalOutput")

    with tile.TileContext(nc) as tc:
        with (
            tc.tile_pool(name="consts", bufs=1) as consts,
            tc.tile_pool(name="persist", bufs=1) as persist,
            tc.tile_pool(name="xqp", bufs=8) as xqp,
            tc.tile_pool(name="qtp", bufs=2) as qtp,
            tc.tile_pool(name="csp", bufs=2) as csp,
            tc.tile_pool(name="ptp", bufs=28) as ptp,
            tc.tile_pool(name="ropep", bufs=1) as ropep,
            tc.tile_pool(name="scp", bufs=4) as scp,
            tc.tile_pool(name="osbp", bufs=3) as osbp,
            tc.tile_pool(name="qkvps", bufs=1, space="PSUM") as qkv_ps,
            tc.tile_pool(name="sps", bufs=2, space="PSUM") as sps,
            tc.tile_pool(name="smallps", bufs=3, space="PSUM") as smallps,
            tc.tile_pool(name="oprojps", bufs=2, space="PSUM") as oprojps,
            tc.tile_pool(name="dram", bufs=1, space="DRAM") as dram,
        ):
            wq_sb = consts.tile([128, G, NDB, H], BF16)
            wk_sb = consts.tile([128, NDB, H], BF16)
            wv_sb = consts.tile([128, NDB, H], BF16)
            wo_sb = consts.tile([128, G, D], BF16)
            mask_sb = consts.tile([128, 128], BF16)
            ident = consts.tile([128, 128], BF16)
            make_identity(nc, ident[:])

            kT = persist.tile([128, T], BF16)
            v_ext = persist.tile([128, NTB, VEXT_STRIDE], BF16)

            rs_in = [dram.tile([n, D], BF16, tag=f"rsw{ch}", name=f"rsw{ch}")
                     for ch, n in enumerate(CHUNK_ROWS)]
            rs_out = [dram.tile([n // N_CORES, D], BF16, tag=f"rso{ch}",
                                name=f"rso{ch}")
                      for ch, n in enumerate(CHUNK_ROWS)]

            nc.vector.memset(v_ext[:, :, 128:129], 1.0)

            # mutable emission state
            state = {
                "xq": {},      # (panel, quarter) -> sbuf tile
                "cs": {},      # panel -> (cos, sin) sbuf tiles
                "qT": {},      # (panel, g) -> roped qT tile [128, 512]
                "pts": {},     # (g, sb) -> (tile, col0) P^T tiles of cur panel
                "oT": {},      # g -> (tile, panel) of cur att panel
                "rs_done": {},
            }

            def fetch_x(p, queues=None):
                qs = queues or [nc.gpsimd] * NQ
                for q in range(NQ):
                    xq = xqp.tile([128, QDB, TP_SIZE], BF16, tag="xq",
                                  name=f"xq{p}_{q}")
                    qs[q].dma_start(
                        out=xq[:],
                        in_=xt_ext[:, p, q * QDB:(q + 1) * QDB, :])
                    state["xq"][(p, q)] = xq

            def fetch_cs(p):
                tsl = slice(p * TP_SIZE, (p + 1) * TP_SIZE)
                cos_sb = csp.tile([H, TP_SIZE], BF16, tag="cos",
                                  name=f"cos{p}")
                sin_sb = csp.tile([H, TP_SIZE], BF16, tag="sin",
                                  name=f"sin{p}")
                nc.gpsimd.dma_start(out=cos_sb[:], in_=cos_ext[:, tsl])
                nc.gpsimd.dma_start(out=sin_sb[:], in_=sin_ext[:, tsl])
                state["cs"][p] = (cos_sb, sin_sb)

            def rope(p, raw, dst):
                """dst = raw*cos + halfswap(raw)*sin for panel p [128,512]."""
                cos_sb, sin_sb = state["cs"][p]
                sw = ropep.tile([128, TP_SIZE], BF16, tag="ropesw",
                                name=f"sw{p}")
                t1 = ropep.tile([128, TP_SIZE], BF16, tag="ropet1",
                                name=f"t1{p}")
                nc.scalar.dma_start(out=sw[0:64, :], in_=raw[64:128, :])
                nc.scalar.dma_start(out=sw[64:128, :], in_=raw[0:64, :])
                nc.vector.tensor_tensor(out=t1[:], in0=raw[:], in1=cos_sb[:],
                                        op=mybir.AluOpType.mult)
                nc.vector.tensor_tensor(out=sw[:], in0=sw[:], in1=sin_sb[:],
                                        op=mybir.AluOpType.mult)
                nc.vector.tensor_tensor(out=dst[:], in0=t1[:], in1=sw[:],
                                        op=mybir.AluOpType.add)

            def qkv_unit(p, which):
                """One QKV output for panel p: 'k' | 'v' | 0..G-1."""
                ps = qkv_ps.tile([128, TP_SIZE], F32, tag="qkv",
                                 name=f"qkv{p}_{which}")
                if which == "k":
                    w = wk_sb
                elif which == "v":
                    w = wv_sb
                else:
                    w = wq_sb[:, which]
                for db in range(NDB):
                    xq = state["xq"][(p, db // QDB)]
                    nc.tensor.matmul(
                        ps[:], w[:, db, :], xq[:, db % QDB, :],
                        start=(db == 0), stop=(db == NDB - 1))
                tsl = slice(p * TP_SIZE, (p + 1) * TP_SIZE)
                if which == "k":
                    nc.scalar.copy(kT[:, tsl], ps[:])
                    rope(p, kT[:, tsl], kT[:, tsl])
                elif which == "v":
                    vraw = scp.tile([128, TP_SIZE], BF16, tag="vraw",
                                    bufs=1, name=f"vraw{p}")
                    nc.scalar.copy(vraw[:], ps[:])
                    for j in range(4):
                        sb = 4 * p + j
                        nc.sync.dma_start(
                            out=v_ext[:, sb, 0:128],
                            in_=vraw[:, j * 128:(j + 1) * 128],
                            transpose=True)
                else:
                    g = which
                    qt = qtp.tile([128, TP_SIZE], BF16, tag=f"qT{g}",
                                  name=f"qT{p}_{g}")
                    nc.vector.tensor_copy(qt[:], ps[:])
                    rope(p, qt[:], qt[:])
                    state["qT"][(p, g)] = qt

            def scores_mm(p, g, sb):
                """One score block matmul + exp + mask for (p, g, sb)."""
                qt = state["qT"][(p, g)]
                jj = sb - 4 * p
                c0 = max(jj, 0) * 128
                w = TP_SIZE - c0
                ps_s = sps.tile([128, TP_SIZE], F32, tag="s",
                                name=f"s{p}_{g}_{sb}")
                nc.tensor.matmul(
                    ps_s[:, 0:w], kT[:, sb * 128:(sb + 1) * 128],
                    qt[:, c0:TP_SIZE], start=True, stop=True)
                pt = ptp.tile([128, TP_SIZE], BF16, tag="pt",
                              name=f"pt{p}_{g}_{sb}")
                nc.scalar.activation(pt[:, 0:w], ps_s[:, 0:w], EXP,
                                     scale=SCALE)
                if jj >= 0:
                    nc.vector.tensor_tensor(
                        out=pt[:, 0:128], in0=pt[:, 0:128], in1=mask_sb[:],
                        op=mybir.AluOpType.mult)
                state["pts"][(p, g, sb)] = (pt, c0)

            def scores_unit(p, g):
                for sb in range(4 * p + 4):
                    scores_mm(p, g, sb)

            def pv_mm(p, g, j):
                """PV matmuls + DVE normalize for t-block j of head g."""
                if g not in state["oT"] or state["oT"][g][1] != p:
                    oT = scp.tile([128, TP_SIZE], BF16, tag=f"oT{g}",
                                  bufs=1, name=f"oT{p}_{g}")
                    state["oT"][g] = (oT, p)
                tb = 4 * p + j
                ps_pv = smallps.tile([128, 132], F32, tag="sm",
                                     name=f"pv{p}_{g}_{j}")
                for sb in range(tb + 1):
                    pt, c0 = state["pts"][(p, g, sb)]
                    lo = j * 128 - c0
                    nc.tensor.matmul(
                        ps_pv[:, 0:129], pt[:, lo:lo + 128],
                        v_ext[:, sb, 0:129],
                        start=(sb == 0), stop=(sb == tb),
                        skip_group_check=True)
                rc = scp.tile([128, 1], F32, tag="rc", bufs=4,
                              name=f"rc{p}_{g}_{j}")
                nc.vector.reciprocal(rc[:], ps_pv[:, 128:129])
                ob = scp.tile([128, 128], BF16, tag="ob",
                              bufs=4, name=f"ob{p}_{g}_{j}")
                nc.vector.tensor_scalar_mul(ob[:], ps_pv[:, 0:128], rc[:])
                return ob

            def pv_tr(p, g, j, ob):
                """PE transpose of normalized block + DVE copy into oT."""
                oT = state["oT"][g][0]
                ps_tr = smallps.tile([128, 128], BF16, tag="sm",
                                     name=f"tr{p}_{g}_{j}")
                nc.tensor.transpose(ps_tr[:], ob[:], ident[:])
                nc.vector.tensor_copy(oT[:, j * 128:(j + 1) * 128],
                                      ps_tr[:])

            def pv_unit(p, g, js=JSEQ):
                """Software-pipelined PV/transpose over t-blocks js."""
                obs = []
                for idx, j in enumerate(js):
                    obs.append((j, pv_mm(p, g, j)))
                    if idx >= 1:
                        jq, obq = obs.pop(0)
                        pv_tr(p, g, jq, obq)
                for jq, obq in obs:
                    pv_tr(p, g, jq, obq)

            def oproj_dq(p, j, dq, force_dve=False):
                """One quarter of o_proj for t-block j of panel p."""
                tb = 4 * p + j
                ch, idx = TB2CHUNK[tb]
                row = idx * 128
                last = p == NTP - 1
                osb = osbp.tile([128, D // 4], BF16, tag="osb",
                                name=f"osb{tb}_{dq}")
                for dp in range(2):
                    od = oprojps.tile([128, 512], F32, tag="od",
                                      name=f"od{tb}_{dq}_{dp}")
                    dc = dq * 2 + dp
                    for g in range(G):
                        nc.tensor.matmul(
                            od[:],
                            state["oT"][g][0][:, j * 128:(j + 1) * 128],
                            wo_sb[:, g, dc * 512:(dc + 1) * 512],
                            start=(g == 0), stop=(g == G - 1),
                            skip_group_check=True)
                    eng = 1 if (last or force_dve) else dq % 2
                    if eng == 0:
                        nc.scalar.copy(
                            osb[:, dp * 512:(dp + 1) * 512], od[:])
                    else:
                        nc.vector.tensor_copy(
                            osb[:, dp * 512:(dp + 1) * 512], od[:])
                q = nc.sync if (last or force_dve or dq % 2 == 1) \
                    else nc.scalar
                q.dma_start(
                    out=rs_in[ch][row:row + 128,
                                  dq * 1024:(dq + 1) * 1024],
                    in_=osb[:])

            def rs_maybe_trigger(tb):
                ch, _ = TB2CHUNK[tb]
                state["rs_done"].setdefault(ch, 0)
                state["rs_done"][ch] += 1
                if state["rs_done"][ch] == len(RS_CHUNKS_TB[ch]):
                    nc.gpsimd.collective_compute(
                        "ReduceScatter",
                        mybir.AluOpType.add,
                        replica_groups=[list(range(N_CORES))],
                        ins=[rs_in[ch].opt()],
                        outs=[rs_out[ch].opt()],
                    )
                    o0, o1 = CHUNK_OUT_OFF[ch], CHUNK_OUT_OFF[ch + 1]
                    nc.gpsimd.dma_start(
                        out=out_ext[o0:o1, :], in_=rs_out[ch][:])

            def oproj_unit(p, j):
                for dq in range(4):
                    oproj_dq(p, j, dq)
                rs_maybe_trigger(4 * p + j)

            # ---- block 0: QKV panel 0, DMA-arrival-ordered ----
            # Startup pieces are 256-512KB so the PE starts ~11us in.
            # Queue sequences (512KB ~ 4.3us at ~119GB/s):
            #  sync:   xq0[0]a, xq0[0]b, xq0[1], xq0[2], xq0[3],
            #          wq2_a, wq2_b, xp1[0], xp1[1]
            #  scalar: wk[0:8], wk[8:16], wk[16:32], wq0_a, wq0_b,
            #          wq3_a, wq3_b, xp1[2], xp1[3]
            #  gpsimd: mask, wv_a, wv_b, cs0, wq1_a, wq1_b, wo, cs1
            xq0 = {}
            for q in range(NQ):
                xq0[q] = xqp.tile([128, QDB, TP_SIZE], BF16, tag="xq",
                                  name=f"xq0_{q}")
                state["xq"][(0, q)] = xq0[q]
            nc.gpsimd.dma_start(out=mask_sb[:], in_=mask_ext[:])
            fetch_cs(0)
            nc.sync.dma_start(out=xq0[0][:, 0:4, :], in_=xt_ext[:, 0, 0:4, :])
            nc.scalar.dma_start(out=wk_sb[:, 0:8], in_=wk_ext[:, 0:8])
            nc.gpsimd.dma_start(out=wv_sb[:, 0:16], in_=wv_ext[:, 0:16])
            nc.sync.dma_start(out=xq0[0][:, 4:QDB, :],
                              in_=xt_ext[:, 0, 4:QDB, :])
            nc.scalar.dma_start(out=wk_sb[:, 8:16], in_=wk_ext[:, 8:16])
            nc.gpsimd.dma_start(out=wv_sb[:, 16:32], in_=wv_ext[:, 16:32])
            nc.sync.dma_start(out=xq0[1][:], in_=xt_ext[:, 0, QDB:2 * QDB, :])
            nc.scalar.dma_start(out=wk_sb[:, 16:32], in_=wk_ext[:, 16:32])
            nc.gpsimd.dma_start(out=wq_sb[:, 3, 0:16], in_=wq_ext[:, 3, 0:16])
            nc.sync.dma_start(out=xq0[2][:],
                              in_=xt_ext[:, 0, 2 * QDB:3 * QDB, :])
            nc.scalar.dma_start(out=wq_sb[:, 0, 0:16], in_=wq_ext[:, 0, 0:16])
            nc.gpsimd.dma_start(out=wq_sb[:, 3, 16:32],
                                in_=wq_ext[:, 3, 16:32])
            nc.sync.dma_start(out=xq0[3][:],
                              in_=xt_ext[:, 0, 3 * QDB:4 * QDB, :])
            nc.scalar.dma_start(out=wq_sb[:, 0, 16:32],
                                in_=wq_ext[:, 0, 16:32])
            nc.sync.dma_start(out=wq_sb[:, 2, 0:16], in_=wq_ext[:, 2, 0:16])
            nc.scalar.dma_start(out=wq_sb[:, 1, 0:16], in_=wq_ext[:, 1, 0:16])
            nc.sync.dma_start(out=wq_sb[:, 2, 16:32], in_=wq_ext[:, 2, 16:32])
            nc.scalar.dma_start(out=wq_sb[:, 1, 16:32],
                                in_=wq_ext[:, 1, 16:32])

            outs0 = ["k", "v", 0, 1, 2, 3]
            pools0 = [sps, sps, qkv_ps, sps, oprojps, oprojps]
            tags0 = ["s", "s", "qkv", "s", "od", "od"]
            ps0 = {}
            for o, pool, tg in zip(outs0, pools0, tags0):
                ps0[o] = pool.tile([128, TP_SIZE], F32, tag=tg,
                                   name=f"qkv0_{o}")
            # (output, quarter) pairs in DMA arrival order; "0a"/"0b"
            # are the two halves of quarter 0 (first k matmuls ~11us).
            PAIR_ORDER = [("k", "0a"), ("k", "0b"), ("v", 0), ("k", 1),
                          ("v", 1), (0, 0), ("k", 2), ("v", 2),
                          (0, 1), (3, 0), (3, 1), ("k", 3), ("v", 3),
                          (0, 2), (0, 3), (3, 2), (3, 3),
                          (1, 0), (1, 1), (2, 0), (2, 1),
                          (1, 2), (1, 3), (2, 2), (2, 3)]
            done = {o: 0 for o in outs0}

            def copyout0(o):
                """Copy-out + rope for output o as soon as it closes."""
                if o == "k":
                    nc.scalar.copy(kT[:, 0:TP_SIZE], ps0["k"][:])
                    rope(0, kT[:, 0:TP_SIZE], kT[:, 0:TP_SIZE])
                elif o == "v":
                    vraw = scp.tile([128, TP_SIZE], BF16, tag="vraw",
                                    bufs=1, name="vraw0")
                    nc.scalar.copy(vraw[:], ps0["v"][:])
                    for j in range(4):
                        nc.sync.dma_start(
                            out=v_ext[:, j, 0:128],
                            in_=vraw[:, j * 128:(j + 1) * 128],
                            transpose=True)
                else:
                    qt = qtp.tile([128, TP_SIZE], BF16, tag=f"qT{o}",
                                  name=f"qT0_{o}")
                    nc.scalar.copy(qt[:], ps0[o][:])
                    rope(0, qt[:], qt[:])
                    state["qT"][(0, o)] = qt

            for o, q in PAIR_ORDER:
                if q == "0a":
                    dbs = range(0, 4)
                elif q == "0b":
                    dbs = range(4, 8)
                else:
                    dbs = range(q * QDB, (q + 1) * QDB)
                w = (wk_sb if o == "k" else
                     wv_sb if o == "v" else wq_sb[:, o])
                for db in dbs:
                    nc.tensor.matmul(
                        ps0[o][:], w[:, db, :],
                        state["xq"][(0, db // QDB)][:, db % QDB, :],
                        start=(done[o] == 0),
                        stop=(done[o] == NDB - 1),
                        skip_group_check=True)
                    done[o] += 1
                if done[o] == NDB:
                    copyout0(o)
            # x panel 1 / cos-sin / wo prefetch on gpsimd, after the
            # latency-critical rope sw DMAs were queued on scalar.
            fetch_cs(1)
            fetch_x(1)
            nc.gpsimd.dma_start(out=wo_sb[:], in_=wo_ext[:])

            # ---- blocks 1..3: att(b-1) + oproj interleaved with QKV(b) --
            for b in range(1, NTP):
                p = b - 1
                qkv = [lambda w=w, b=b: qkv_unit(b, w)
                       for w in ["k", "v", 0, 1, 2, 3]]
                att = [lambda p=p: scores_unit(p, 0)]
                for g in range(G - 1):
                    def pv_sc(g=g, p=p):
                        pv_unit(p, g)
                        scores_unit(p, g + 1)
                    att.append(pv_sc)
                # g3 + oproj tail (PV mm / PE transpose / oproj
                # staggered); for p==2 defer tb11/tb8 oproj to block 4.
                oj = JSEQ if p < 2 else JSEQ[:2]
                ts = {"obs": []}

                def pv3_push(jj, p=p, ts=ts):
                    ts["obs"].append((jj, pv_mm(p, G - 1, jj)))

                def tr_pop(p=p, ts=ts):
                    jj, ob = ts["obs"].pop(0)
                    pv_tr(p, G - 1, jj, ob)

                att.append(lambda: pv3_push(JSEQ[0]))
                att.append(lambda: (pv3_push(JSEQ[1]), tr_pop()))
                att.append(lambda p=p, oj=oj: (pv3_push(JSEQ[2]), tr_pop(),
                                               oproj_unit(p, oj[0])))
                att.append(lambda p=p, oj=oj: (pv3_push(JSEQ[3]), tr_pop(),
                                               oproj_unit(p, oj[1])))
                if len(oj) == 4:
                    att.append(lambda p=p: (tr_pop(),
                                            oproj_unit(p, JSEQ[2])))
                    att.append(lambda p=p: oproj_unit(p, JSEQ[3]))
                else:
                    att.append(lambda: tr_pop())
                n_u = max(len(att), len(qkv))
                for i in range(n_u):
                    if i < len(att):
                        att[i]()
                    if i < len(qkv):
                        qkv[i]()
                    if i == 0 and b + 1 < NTP:
                        fetch_cs(b + 1)
                    if i == 2 and b + 1 < NTP:
                        fetch_x(b + 1)

            # ---- block 4: att(panel 3) + deferred oproj(p2) ----
            # ScalarE is the bottleneck in the score phase (64 exps
            # ~41us vs 17us of score MMs), so filler PE work (deferred
            # p2 oproj quarters, then pv units) is interleaved into
            # the exp-gated score stream at ~per-3-MMs granularity.
            p = 3
            fillers = []
            for j in (JSEQ[2], JSEQ[3]):   # tb11 then tb8
                for dq in range(4):
                    fillers.append(
                        lambda j=j, dq=dq: oproj_dq(2, j, dq,
                                                    force_dve=True))
                fillers.append(lambda j=j: rs_maybe_trigger(8 + j))
            fill_i = 0
            mm_cnt = 0
            for g in range(G):
                if g >= 1:
                    # pv(g-1) pieces become fillers consumable during
                    # sc(g) (its pts are complete by now).
                    st = {"obs": []}

                    def mmf(j, g=g, st=st):
                        st["obs"].append((j, pv_mm(p, g - 1, j)))

                    def trf(g=g, st=st):
                        j, ob = st["obs"].pop(0)
                        pv_tr(p, g - 1, j, ob)

                    fillers.append(lambda f=mmf: f(JSEQ[0]))
                    fillers.append(lambda f=mmf, t=trf: (f(JSEQ[1]), t()))
                    fillers.append(lambda f=mmf, t=trf: (f(JSEQ[2]), t()))
                    fillers.append(lambda f=mmf, t=trf: (f(JSEQ[3]), t()))
                    fillers.append(lambda t=trf: t())
                for sb in range(4 * p + 4):
                    scores_mm(p, g, sb)
                    mm_cnt += 1
                    step = 2 if fill_i < 10 else 3
                    if mm_cnt % step == 0 and fill_i < len(fillers):
                        fillers[fill_i]()
                        fill_i += 1
            while fill_i < len(fillers):
                fillers[fill_i]()
                fill_i += 1
            # tail: pv(g3) mm / tr / oproj staggered
            obs3 = []
            obs3.append((JSEQ[0], pv_mm(p, G - 1, JSEQ[0])))
            obs3.append((JSEQ[1], pv_mm(p, G - 1, JSEQ[1])))
            jj, ob = obs3.pop(0)
            pv_tr(p, G - 1, jj, ob)
            obs3.append((JSEQ[2], pv_mm(p, G - 1, JSEQ[2])))
            jj, ob = obs3.pop(0)
            pv_tr(p, G - 1, jj, ob)
            oproj_unit(p, JSEQ[0])
            obs3.append((JSEQ[3], pv_mm(p, G - 1, JSEQ[3])))
            jj, ob = obs3.pop(0)
            pv_tr(p, G - 1, jj, ob)
            oproj_unit(p, JSEQ[1])
            jj, ob = obs3.pop(0)
            pv_tr(p, G - 1, jj, ob)
            oproj_unit(p, JSEQ[2])
            oproj_unit(p, JSEQ[3])

    nc.compile()
    return nc


def get_nc():
    if "nc" not in _NC_CACHE:
        _NC_CACHE["nc"] = _build_nc()
    return _NC_CACHE["nc"]


def make_in_maps(x, positions, w_q, w_k, w_v, w_o):
    """Host-side sharding + RoPE table / mask precompute."""
    x = np.ascontiguousarray(np.asarray(x, np.float32))
    positions = np.asarray(positions)

    half = H // 2
    inv_freq = 1.0 / (THETA ** (np.arange(half, dtype=np.float32) / half))
    ang = positions.astype(np.float32)[:, None] * inv_freq[None, :]  # [T, 64]
    cos = np.cos(ang)   # [T, 64]
    sin = np.sin(ang)
    cos_t = np.empty((H, T), np.float32)
    sin_t = np.empty((H, T), np.float32)
    cos_t[0:half] = cos.T
    cos_t[half:] = cos.T
    sin_t[0:half] = -sin.T
    sin_t[half:] = sin.T
    cos_t = cos_t.astype(ml_dtypes.bfloat16)
    sin_t = sin_t.astype(ml_dtypes.bfloat16)

    # mask[s, t] = 1 if s <= t (lower-left of P^T allowed region)
    idx = np.arange(128)
    maskp = (idx[:, None] <= idx[None, :]).astype(ml_dtypes.bfloat16)

    xt = x.astype(ml_dtypes.bfloat16).T  # [D, T]
    xt4 = np.ascontiguousarray(
        xt.reshape(NDB, 128, NTP, TP_SIZE).transpose(1, 2, 0, 3))
    w_q = np.asarray(w_q, np.float32).reshape(D, NH, H).astype(
        ml_dtypes.bfloat16)
    w_k = np.asarray(w_k, np.float32).reshape(D, KH, H).astype(
        ml_dtypes.bfloat16)
    w_v = np.asarray(w_v, np.float32).reshape(D, KH, H).astype(
        ml_dtypes.bfloat16)
    w_o = np.asarray(w_o, np.float32).reshape(NH, H, D).astype(
        ml_dtypes.bfloat16)

    def blk(w):
        """[D, n] -> [128, NDB, n] with row d = a*128 + p."""
        return np.ascontiguousarray(
            w.reshape(NDB, 128, -1).transpose(1, 0, 2))

    in_maps = []
    for c in range(N_CORES):
        # wq g-major: [128, G, NDB, H] so each head's chunk is contiguous
        wq_c = w_q[:, G * c:G * (c + 1), :]            # [D, G, H]
        wq_blk = np.ascontiguousarray(
            wq_c.reshape(NDB, 128, G, H).transpose(1, 2, 0, 3))
        in_maps.append({
            "xt": xt4,
            "wq": wq_blk,
            "wk": blk(w_k[:, c, :]),
            "wv": blk(w_v[:, c, :]),
            "wo": np.ascontiguousarray(
                w_o[G * c:G * (c + 1)].reshape(G, 128, D)
                .transpose(1, 0, 2)),
            "cos_t": cos_t,
            "sin_t": sin_t,
            "maskp": maskp,
        })
    return in_maps


def assemble_output(results):
    """results: list of 8 per-core dicts with 'out' [T//8, D] bf16.

    Chunk ch rows are packed t-block-list-major; the RS gave core c
    chunk-tile rows [c*k, (c+1)*k) where k = chunk_rows/8.
    """
    out = np.empty((T, D), np.float32)
    for c in range(N_CORES):
        o = np.asarray(results[c]["out"], np.float32)
        for ch, tbs in enumerate(RS_CHUNKS_TB):
            k = CHUNK_ROWS[ch] // N_CORES
            piece = o[CHUNK_OUT_OFF[ch]:CHUNK_OUT_OFF[ch + 1]]
            for r in range(k):
                cr = c * k + r
                tb = tbs[cr // 128]
                out[tb * 128 + (cr % 128)] = piece[r]
    return out


def kernel(x, positions, w_q, w_k, w_v, w_o):
    from concourse.bass_utils import run_bass_kernel_spmd

    _enable_ldw_opt()
    nc = get_nc()
    in_maps = make_in_maps(x, positions, w_q, w_k, w_v, w_o)
    res = run_bass_kernel_spmd(nc, in_maps, core_ids=list(range(N_CORES)))
    return assemble_output(res.results)
